# revision 9
# baseline (speedup 1.0000x reference)
"""Trainium2 Bass kernel for nn_Attention_5514738008849.

Dense transformer attention block with axial rotary embeddings:
  x:(8,1024,1024) -> qkv -> rope(q,k) -> softmax(qk^T/sqrt(d)) v -> proj+bias

Sharding: pure data-parallel over batch B=8 across the 8 NeuronCores (one
batch element per core, full weights replicated). No collectives.

Per-core dataflow, software-pipelined at head-pair granularity so the PE
stream never drains and the ACT engine mostly runs the exp chain:
  - seg0: V = x W_v (token-major, fp32r, 512-wide), Q^T/K^T for head-pair 0
  - segs 1..8: QKV+rotary for head-pair hp interleaved (as PE filler) with
    attention for the two heads of hp-1
  - rotary: pair-shuffle via signed-permutation matmul + cos/sin on DVE,
    Q/K evacuations on DVE, V evacuations on ACT (idle early); rotated Q/K
    stored bf16
  - logits^T[k,q] per head (bf16 operands); exp on ACT (scale=1/8) -> bf16
  - AV transposed: stationary = exp tile [128k,128q], moving = V|ones bf16
    [128k,65] => out[q-block, d|rowsum]; costs 65 PE cycles/matmul instead
    of 512 (cost model charges the moving free size only)
  - softmax normalize fused into the PSUM evacuation: DVE tensor_scalar by
    the per-(q,qb) reciprocal row-sum, writing bf16 token-major attn
  - attn transposed to c-major via the DMA XBAR (bf16, off the compute
    engines) per head-pair, overlapped with attention
  - proj: bf16 attn^T x bf16 w_proj; bias added on the DVE evacuation via a
    host-replicated [128,1024] bias tile; fp32 output
"""

import os
import sys

sys.path.insert(0, "/opt/trn_rl_repo")

# This kernel needs the axon-tunneled NeuronCores. A JAX_PLATFORMS=cpu pin
# (used by some harnesses for the jax reference) would prevent the axon
# backend from registering; clearing it here is a no-op when jax has already
# initialized and restores device visibility when it hasn't.
if os.environ.get("JAX_PLATFORMS", "") not in ("", None):
    if "axon" not in os.environ["JAX_PLATFORMS"]:
        os.environ.pop("JAX_PLATFORMS", None)

import numpy as np

import concourse.bass as bass
import concourse.bacc as bacc_mod
import concourse.mybir as mybir
from concourse.bass_utils import run_bass_kernel_spmd
from concourse.tile import TileContext

B, N, C = 8, 1024, 1024
H, D = 16, 64          # heads, head dim
ROT = 32               # rotary dims per head (head_dim // 2)
FH = FW = 32           # token grid for axial rope
NCORES = 8
F32 = mybir.dt.float32
F32R = mybir.dt.float32r
BF16 = mybir.dt.bfloat16


def _host_tables():
    """Rotary cos/sin in d-major (dim-on-partition) layout + shuffle matrix."""
    dim_r = D // 4                                    # 16
    base = np.linspace(1.0, (FH * FW) / 2.0, dim_r // 2) * np.pi   # (8,)

    def axis_freqs(n):
        pos = np.linspace(-1.0, 1.0, n)
        f = pos[:, None] * base[None, :]              # (n, 8)
        return np.repeat(f, 2, axis=-1)               # (n, 16)

    fH = np.broadcast_to(axis_freqs(FH)[:, None, :], (FH, FW, dim_r))
    fW = np.broadcast_to(axis_freqs(FW)[None, :, :], (FH, FW, dim_r))
    freqs = np.concatenate([fH, fW], axis=-1).reshape(N, ROT)      # (1024, 32)

    # d-major table for one 128-partition block = two heads:
    # rows 0-31 rot (head even), 32-63 pass, 64-95 rot (head odd), 96-127 pass
    cos_d = np.ones((128, N), np.float32)
    sin_d = np.zeros((128, N), np.float32)
    ct = np.cos(freqs).T.astype(np.float32)           # (32, 1024)
    st = np.sin(freqs).T.astype(np.float32)
    cos_d[0:32] = ct
    cos_d[64:96] = ct
    sin_d[0:32] = st
    sin_d[64:96] = st

    # signed permutation: shuf[2i] = -q[2i+1], shuf[2i+1] = q[2i] on rot rows
    pshuf = np.zeros((128, 128), np.float32)
    for off in (0, 64):
        for i in range(ROT // 2):
            r0, r1 = off + 2 * i, off + 2 * i + 1
            pshuf[r1, r0] = -1.0                      # out[r0] = -in[r1]
            pshuf[r0, r1] = 1.0                       # out[r1] = +in[r0]

    return cos_d, sin_d, pshuf


def _build_program():
    nc = bacc_mod.Bacc()
    xt_h = nc.declare_dram_parameter("xt", [C, N], F32, isOutput=False)
    wqkv_h = nc.declare_dram_parameter("w_qkv", [C, 3 * C], F32, isOutput=False)
    wproj_h = nc.declare_dram_parameter("w_proj", [C, C], F32, isOutput=False)
    bb_h = nc.declare_dram_parameter("b_bcast", [128, C], F32, isOutput=False)
    cos_h = nc.declare_dram_parameter("cos_d", [128, N], F32, isOutput=False)
    sin_h = nc.declare_dram_parameter("sin_d", [128, N], F32, isOutput=False)
    pshuf_h = nc.declare_dram_parameter("pshuf", [128, 128], F32, isOutput=False)
    out_h = nc.declare_dram_parameter("out", [N, C], F32, isOutput=True)
    guard_h = nc.declare_dram_parameter("guard", [128, 64], BF16, isOutput=True)

    def f32r(ap):
        return ap.bitcast(F32R)

    MM = nc.tensor.matmul
    EXP = mybir.ActivationFunctionType.Exp
    MULT = mybir.AluOpType.mult

    with nc.allow_low_precision(reason="fp32r/bf16 operands"), \
         TileContext(nc) as tc, \
         tc.tile_pool(name="consts", bufs=1) as consts, \
         tc.tile_pool(name="persist", bufs=1) as persist, \
         tc.tile_pool(name="ep", bufs=3) as ep, \
         tc.tile_pool(name="plg", bufs=1, space="PSUM") as plg, \
         tc.tile_pool(name="pav", bufs=2, space="PSUM") as pav:

        cos_sb = consts.tile([128, N], F32)
        sin_sb = consts.tile([128, N], F32)
        pshuf_sb = consts.tile([128, 128], F32)
        bb_sb = consts.tile([128, C], F32)

        qrot = persist.tile([128, 8, N], BF16)     # Q_rot^T  (d-major, bf16)
        krot = persist.tile([128, 8, N], BF16)
        vext = persist.tile([128, 8, H, 65], BF16)  # V | ones per tok-block
        atok = persist.tile([128, 8, 8, 128], BF16)  # [q, hp, qb, c-in-hp]
        # one whole tile per head-pair: the XBAR transpose requires an
        # offset-0 (whole-tile) output AP
        attn_Ts = [persist.tile([128, 8, 128], BF16, name=f"attn_T{i}")
                   for i in range(8)]
        recip = persist.tile([128, H, 8], F32)       # 1/rowsum per (q, h, qb)

        # ones column of vext (softmax denominator accumulator)
        nc.gpsimd.memset(vext[:, :, :, 64:65], 1.0)

        # ---------------- attention emission helpers ----------------
        def lg_unit(h, kt, estore):
            hp, r0 = h // 2, (h % 2) * 64
            lg_t = plg.tile([128, N], F32, tag="lg", name=f"lg{h}_{kt}")
            for qc in range(2):
                MM(lg_t[:, qc * 512:(qc + 1) * 512],
                   krot[r0:r0 + 64, hp, kt * 128:(kt + 1) * 128],
                   qrot[r0:r0 + 64, hp, qc * 512:(qc + 1) * 512],
                   start=True, stop=True)
            e_t = ep.tile([128, N], BF16, tag="e", name=f"e{h}_{kt}")
            nc.scalar.activation(e_t, lg_t, EXP, scale=0.125)
            estore[kt] = e_t

        def av_unit(h, kt, estore, av_ts):
            # start=True zeroes the whole PSUM bank, so only the first group
            # per bank (qb 0 and 4) may use it; later groups accumulate onto
            # the freshly-zeroed bank.
            e_t = estore[kt]
            for qb in range(8):
                t = av_ts[qb // 4]
                j = qb % 4
                MM(t[:, j * 128:j * 128 + 65],
                   e_t[:, qb * 128:(qb + 1) * 128],
                   vext[:, kt, h, :],
                   start=(kt == 0 and j == 0), stop=(kt == 7),
                   skip_group_check=True)

        def norm_unit(h, av_ts):
            hp, cr0 = h // 2, (h % 2) * 64
            for half in range(2):
                t = av_ts[half].rearrange("p (a b) -> p a b", a=4)
                nc.vector.reciprocal(
                    recip[:, h, half * 4:half * 4 + 4], t[:, :, 64])
            for qb in range(8):
                t = av_ts[qb // 4].rearrange("p (a b) -> p a b", a=4)
                nc.vector.tensor_scalar(
                    atok[:, hp, qb, cr0:cr0 + 64],
                    t[:, qb % 4, 0:64],
                    recip[:, h, qb:qb + 1], None, MULT)

        def attention_steps(h):
            """List of callables: steps 0..7 emit logits(kt) [+ av(kt-1)];
            the final step emits av(7) and the normalize."""
            state = {}

            def step(kt, h=h, state=state):
                if kt == 0:
                    state["av"] = [
                        pav.tile([128, 512], F32, tag="av", name=f"av{h}_{i}")
                        for i in range(2)]
                    state["e"] = {}
                lg_unit(h, kt, state["e"])
                if kt > 0:
                    av_unit(h, kt - 1, state["e"], state["av"])

            steps = [(lambda kt=kt: step(kt)) for kt in range(8)]

            def last(h=h, state=state):
                av_unit(h, 7, state["e"], state["av"])
                norm_unit(h, state["av"])
            steps.append(last)
            return steps

        # ---------------- main pipelined phase ----------------
        with tc.tile_pool(name="xtp", bufs=1) as xtp, \
             tc.tile_pool(name="wqk", bufs=3) as wqk, \
             tc.tile_pool(name="wvp", bufs=1) as wvp, \
             tc.tile_pool(name="qsb", bufs=2) as qsbp, \
             tc.tile_pool(name="tmp", bufs=2) as tmpp, \
             tc.tile_pool(name="pq", bufs=1, space="PSUM") as pq, \
             tc.tile_pool(name="psh", bufs=1, space="PSUM") as psh:

            # -- DMA issue (emission order = per-queue order) --
            wv_cell = {}

            def fetch_wv(og):
                t = wvp.tile([128, 8, 512], F32, tag="wv", name=f"wv{og}")
                nc.gpsimd.dma_start(
                    out=f32r(t),
                    in_=f32r(wqkv_h[:, 2048 + og * 512:2048 + (og + 1) * 512]
                             .rearrange("(a p) c -> p a c", p=128)))
                wv_cell[og] = t

            fetch_wv(0)
            xT = xtp.tile([128, 8, N], F32)
            for kb in range(8):
                nc.gpsimd.dma_start(
                    out=f32r(xT[:, kb, :]),
                    in_=f32r(xt_h[kb * 128:(kb + 1) * 128, :]))

            wq_ts, wk_ts = {}, {}

            def fetch_wqk(hp):
                for d, store, c0 in ((0, wq_ts, 0), (1, wk_ts, C)):
                    t = wqk.tile([128, 8, 128], F32, tag=f"w{d}",
                                 name=f"w{d}_{hp}")
                    nc.sync.dma_start(
                        out=f32r(t),
                        in_=f32r(wqkv_h[:, c0 + hp * 128:c0 + (hp + 1) * 128]
                                 .rearrange("(a p) c -> p a c", p=128)))
                    store[hp] = t

            fetch_wqk(0)
            nc.sync.dma_start(out=f32r(pshuf_sb), in_=f32r(pshuf_h[:, :]))
            nc.sync.dma_start(out=cos_sb, in_=cos_h[:, :])
            nc.sync.dma_start(out=sin_sb, in_=sin_h[:, :])
            fetch_wqk(1)
            fetch_wqk(2)

            # -- QKV emission units --
            def qk_units(hp):
                units = []
                for d, wst, dst in ((0, wq_ts, qrot), (1, wk_ts, krot)):
                    cell = {}

                    def mm_kb(kb, d=d, wst=wst, cell=cell, hp=hp):
                        if kb == 0:
                            cell["ps"] = pq.tile([128, N], F32, tag="qk",
                                                 name=f"qk{d}_{hp}")
                        ps = cell["ps"]
                        for qc in range(2):
                            MM(ps[:, qc * 512:(qc + 1) * 512],
                               f32r(wst[hp][:, kb, :]),
                               f32r(xT[:, kb, qc * 512:(qc + 1) * 512]),
                               start=(kb == 0), stop=(kb == 7))

                    units.extend(
                        (lambda kb=kb, f=mm_kb: f(kb)) for kb in range(8))

                    def rot(d=d, dst=dst, cell=cell, hp=hp):
                        q_sb = qsbp.tile([128, N], F32, tag="q",
                                         name=f"q{d}_{hp}")
                        nc.vector.tensor_copy(f32r(q_sb), cell["ps"])
                        sh_ps = psh.tile([128, N], F32, tag="sh",
                                         name=f"sh{d}_{hp}")
                        for qc in range(2):
                            MM(sh_ps[:, qc * 512:(qc + 1) * 512],
                               f32r(pshuf_sb),
                               f32r(q_sb[:, qc * 512:(qc + 1) * 512]),
                               start=True, stop=True)
                        tm = tmpp.tile([128, N], F32, tag="t",
                                       name=f"t{d}_{hp}")
                        nc.vector.tensor_mul(tm, sh_ps, sin_sb)
                        nc.vector.tensor_mul(dst[:, hp, :], q_sb, cos_sb)
                        nc.vector.tensor_add(dst[:, hp, :], dst[:, hp, :], tm)
                    units.append(rot)
                if hp + 3 < 8:
                    units.append(lambda hp=hp: fetch_wqk(hp + 3))
                return units

            def v_block(og, tb_pair):
                """V for tok-blocks (2*tb_pair, 2*tb_pair+1), heads og*8..+8."""
                v_ps = pq.tile([128, N], F32, tag="qk", name=f"v{og}_{tb_pair}")
                for t in range(2):
                    tb = 2 * tb_pair + t
                    for kb in range(8):
                        MM(v_ps[:, t * 512:(t + 1) * 512],
                           f32r(xT[:, kb, tb * 128:(tb + 1) * 128]),
                           f32r(wv_cell[og][:, kb, :]),
                           start=(kb == 0), stop=(kb == 7))
                nc.scalar.copy(
                    vext[:, 2 * tb_pair:2 * tb_pair + 2,
                         og * 8:(og + 1) * 8, 0:64],
                    v_ps.rearrange("p (a b c) -> p a b c", a=2, b=8))

            def transpose_hp(hp):
                # tracked guard read: the XBAR transpose's input dependency is
                # not reliably honored; an ordinary (tracked) DMA on the same
                # SP queue forces the sequencer to wait for the norm writes
                # before the transpose issues.
                nc.sync.dma_start(out=guard_h[:, hp * 8:(hp + 1) * 8],
                                  in_=atok[:, hp, 7, 120:128])
                nc.sync.dma_start(out=attn_Ts[hp],
                                  in_=atok[:, hp, :, :], transpose=True)

            # -- seg0: V(heads 0-7), Q/K(hp0), V(heads 8-15); no attention --
            for tb_pair in range(4):
                v_block(0, tb_pair)
            for u in qk_units(0):
                u()
            fetch_wv(1)
            for tb_pair in range(4):
                v_block(1, tb_pair)

            # -- segs 1..8: attention heads of hp-1, filled with QKV(hp) --
            for seg in range(1, 9):
                fillers = qk_units(seg) if seg < 8 else []
                fi = 0
                for h in (2 * (seg - 1), 2 * (seg - 1) + 1):
                    for step_fn in attention_steps(h):
                        step_fn()
                        for _ in range(2):
                            if fi < len(fillers):
                                fillers[fi]()
                                fi += 1
                while fi < len(fillers):
                    fillers[fi]()
                    fi += 1
                transpose_hp(seg - 1)

        # ---------------- proj + bias + output ----------------
        with tc.tile_pool(name="wpp", bufs=1) as wpp, \
             tc.tile_pool(name="ysb", bufs=2) as ysbp, \
             tc.tile_pool(name="psy", bufs=2, space="PSUM") as psy:
            wp_sb = wpp.tile([128, 8, C], BF16)
            nc.gpsimd.dma_start(
                out=wp_sb,
                in_=wproj_h.rearrange("(a p) c -> p a c", p=128))
            nc.sync.dma_start(out=bb_sb, in_=bb_h[:, :])

            for tb in range(8):
                y_ps = psy.tile([128, C], F32, tag="y", name=f"y{tb}")
                for cb in range(8):
                    for oc in range(2):
                        MM(y_ps[:, oc * 512:(oc + 1) * 512],
                           attn_Ts[cb][:, tb, :],
                           wp_sb[:, cb, oc * 512:(oc + 1) * 512],
                           start=(cb == 0), stop=(cb == 7))
                y_sb = ysbp.tile([128, C], F32, tag="ysb", name=f"ysb{tb}")
                nc.vector.tensor_add(y_sb, y_ps, bb_sb)
                nc.sync.dma_start(out=out_h[tb * 128:(tb + 1) * 128, :],
                                  in_=y_sb)

    nc.finalize()
    return nc


_PROGRAM = None


def kernel(x, w_qkv, w_proj, b_proj):
    global _PROGRAM
    if _PROGRAM is None:
        _PROGRAM = _build_program()
    nc = _PROGRAM

    cos_d, sin_d, pshuf = _host_tables()
    shared = {
        "w_qkv": np.ascontiguousarray(w_qkv, np.float32),
        "w_proj": np.ascontiguousarray(w_proj, np.float32),
        "b_bcast": np.ascontiguousarray(
            np.broadcast_to(np.asarray(b_proj, np.float32).reshape(1, C),
                            (128, C))),
        "cos_d": cos_d,
        "sin_d": sin_d,
        "pshuf": pshuf,
    }
    in_maps = [
        {"xt": np.ascontiguousarray(np.asarray(x[b], np.float32).T), **shared}
        for b in range(NCORES)
    ]
    res = run_bass_kernel_spmd(nc, in_maps, core_ids=list(range(NCORES)))
    return np.stack([res.results[b]["out"] for b in range(NCORES)], axis=0)


if __name__ == "__main__":
    xs = np.random.randn(B, N, C).astype(np.float32)
    wq = (np.random.randn(C, 3 * C) / np.sqrt(C)).astype(np.float32)
    wp = (np.random.randn(C, C) / np.sqrt(C)).astype(np.float32)
    bp = (np.random.randn(C) * 0.01).astype(np.float32)
    out = kernel(x=xs, w_qkv=wq, w_proj=wp, b_proj=bp)
    print(out.shape, out.dtype)


# revision 10
# speedup vs baseline: 1.3765x; 1.3765x over previous
"""Trainium2 Bass kernel for nn_Attention_5514738008849.

Dense transformer attention block with axial rotary embeddings:
  x:(8,1024,1024) -> qkv -> rope(q,k) -> softmax(qk^T/sqrt(d)) v -> proj+bias

Sharding: pure data-parallel over batch B=8 across the 8 NeuronCores (one
batch element per core, full weights replicated). No collectives.

Per-core dataflow, software-pipelined so the PE stream never drains (the PE
executes its stream in order, so emission order IS the schedule):
  - seg0: V(heads 0-7) + Q/K(head-pair 0); V(heads 8-15) spread over segs 1-4
  - segs 1..8: attention for the heads of hp-1, with QKV+rotary work for
    head-pair hp interleaved between attention steps as PE filler
  - attention per head: logits(kt) on a double-buffered [128,1024] PSUM pair;
    exp on ACT (scale=1/8) -> bf16; the AV matmuls lag the logits stream by
    two steps so they never wait on an exp in flight
  - AV transposed: stationary = exp tile [128k,128q], moving = V|ones bf16
    [128k,65] => out[q-block, d|rowsum]; 65 PE cycles/matmul instead of 512
    (only the first accumulation group per PSUM bank may use start=True:
    start zeroes the whole bank)
  - softmax normalize fused into the PSUM evacuation: DVE tensor_scalar by
    the per-(q,qb) reciprocal row-sum, writing bf16 token-major attn
  - attn transposed to c-major via the DMA XBAR (bf16, off the compute
    engines) per head-pair; a tracked guard DMA on the same queue forces the
    XBAR (whose input deps aren't honored) to wait for the normalize
  - proj: bf16 attn^T x bf16 w_proj (host-cast); bias added on the DVE
    evacuation via a host-replicated [128,1024] bias tile; fp32 output
"""

import os
import sys

sys.path.insert(0, "/opt/trn_rl_repo")

# This kernel needs the axon-tunneled NeuronCores. A JAX_PLATFORMS=cpu pin
# (used by some harnesses for the jax reference) would prevent the axon
# backend from registering; clearing it here is a no-op when jax has already
# initialized and restores device visibility when it hasn't.
if os.environ.get("JAX_PLATFORMS", "") not in ("", None):
    if "axon" not in os.environ["JAX_PLATFORMS"]:
        os.environ.pop("JAX_PLATFORMS", None)

from collections import deque

import numpy as np

import concourse.bass as bass
import concourse.bacc as bacc_mod
import concourse.mybir as mybir
from concourse.bass_utils import run_bass_kernel_spmd
from concourse.tile import TileContext

B, N, C = 8, 1024, 1024
H, D = 16, 64          # heads, head dim
ROT = 32               # rotary dims per head (head_dim // 2)
FH = FW = 32           # token grid for axial rope
NCORES = 8
F32 = mybir.dt.float32
F32R = mybir.dt.float32r
BF16 = mybir.dt.bfloat16


def _host_tables():
    """Rotary cos/sin in d-major (dim-on-partition) layout + shuffle matrix."""
    dim_r = D // 4                                    # 16
    base = np.linspace(1.0, (FH * FW) / 2.0, dim_r // 2) * np.pi   # (8,)

    def axis_freqs(n):
        pos = np.linspace(-1.0, 1.0, n)
        f = pos[:, None] * base[None, :]              # (n, 8)
        return np.repeat(f, 2, axis=-1)               # (n, 16)

    fH = np.broadcast_to(axis_freqs(FH)[:, None, :], (FH, FW, dim_r))
    fW = np.broadcast_to(axis_freqs(FW)[None, :, :], (FH, FW, dim_r))
    freqs = np.concatenate([fH, fW], axis=-1).reshape(N, ROT)      # (1024, 32)

    # d-major table for one 128-partition block = two heads:
    # rows 0-31 rot (head even), 32-63 pass, 64-95 rot (head odd), 96-127 pass
    cos_d = np.ones((128, N), np.float32)
    sin_d = np.zeros((128, N), np.float32)
    ct = np.cos(freqs).T.astype(np.float32)           # (32, 1024)
    st = np.sin(freqs).T.astype(np.float32)
    cos_d[0:32] = ct
    cos_d[64:96] = ct
    sin_d[0:32] = st
    sin_d[64:96] = st

    # signed permutation: shuf[2i] = -q[2i+1], shuf[2i+1] = q[2i] on rot rows
    pshuf = np.zeros((128, 128), np.float32)
    for off in (0, 64):
        for i in range(ROT // 2):
            r0, r1 = off + 2 * i, off + 2 * i + 1
            pshuf[r1, r0] = -1.0                      # out[r0] = -in[r1]
            pshuf[r0, r1] = 1.0                       # out[r1] = +in[r0]

    return cos_d, sin_d, pshuf


def _build_program():
    nc = bacc_mod.Bacc()
    xt_h = nc.declare_dram_parameter("xt", [C, N], F32, isOutput=False)
    wqkv_h = nc.declare_dram_parameter("w_qkv", [C, 3 * C], F32, isOutput=False)
    wpb_h = nc.declare_dram_parameter("w_projb", [C, C], BF16, isOutput=False)
    bb_h = nc.declare_dram_parameter("b_bcast", [128, C], F32, isOutput=False)
    cos_h = nc.declare_dram_parameter("cos_d", [128, N], F32, isOutput=False)
    sin_h = nc.declare_dram_parameter("sin_d", [128, N], F32, isOutput=False)
    pshuf_h = nc.declare_dram_parameter("pshuf", [128, 128], F32, isOutput=False)
    out_h = nc.declare_dram_parameter("out", [N, C], F32, isOutput=True)
    guard_h = nc.declare_dram_parameter("guard", [128, 64], BF16, isOutput=True)

    def f32r(ap):
        return ap.bitcast(F32R)

    MM = nc.tensor.matmul
    EXP = mybir.ActivationFunctionType.Exp
    MULT = mybir.AluOpType.mult

    with nc.allow_low_precision(reason="fp32r/bf16 operands"), \
         TileContext(nc) as tc, \
         tc.tile_pool(name="consts", bufs=1) as consts, \
         tc.tile_pool(name="persist", bufs=1) as persist:

        cos_sb = consts.tile([128, N], F32)
        sin_sb = consts.tile([128, N], F32)
        pshuf_sb = consts.tile([128, 128], F32)
        bb_sb = consts.tile([128, C], F32)

        qrot = persist.tile([128, 8, N], BF16)     # Q_rot^T  (d-major, bf16)
        krot = persist.tile([128, 8, N], BF16)
        vext = persist.tile([128, 8, H, 65], BF16)  # V | ones per tok-block
        atok = persist.tile([128, 8, 8, 128], BF16)  # [q, hp, qb, c-in-hp]
        # one whole tile per head-pair: the XBAR transpose requires an
        # offset-0 (whole-tile) output AP
        attn_Ts = [persist.tile([128, 8, 128], BF16, name=f"attn_T{i}")
                   for i in range(8)]
        recip = persist.tile([128, H, 8], F32)       # 1/rowsum per (q, h, qb)

        # ones column of vext (softmax denominator accumulator)
        nc.gpsimd.memset(vext[:, :, :, 64:65], 1.0)

        with tc.tile_pool(name="ep", bufs=4) as ep, \
             tc.tile_pool(name="plg", bufs=2, space="PSUM") as plg, \
             tc.tile_pool(name="pav", bufs=2, space="PSUM") as pav, \
             tc.tile_pool(name="xtp", bufs=1) as xtp, \
             tc.tile_pool(name="wqk", bufs=3) as wqk, \
             tc.tile_pool(name="wvp", bufs=1) as wvp, \
             tc.tile_pool(name="qsb", bufs=2) as qsbp, \
             tc.tile_pool(name="tmp", bufs=2) as tmpp, \
             tc.tile_pool(name="pqk", bufs=1, space="PSUM") as pqk, \
             tc.tile_pool(name="psh", bufs=1, space="PSUM") as psh:

            # ---------------- DMA issue (emission order = queue order) ----
            wv_cell = {}

            def fetch_wv(og):
                t = wvp.tile([128, 8, 512], F32, tag="wv", name=f"wv{og}")
                nc.gpsimd.dma_start(
                    out=f32r(t),
                    in_=f32r(wqkv_h[:, 2048 + og * 512:2048 + (og + 1) * 512]
                             .rearrange("(a p) c -> p a c", p=128)))
                wv_cell[og] = t

            fetch_wv(0)
            xT = xtp.tile([128, 8, N], F32)
            for kb in range(8):
                nc.gpsimd.dma_start(
                    out=f32r(xT[:, kb, :]),
                    in_=f32r(xt_h[kb * 128:(kb + 1) * 128, :]))

            wq_ts, wk_ts = {}, {}

            def fetch_wqk(hp):
                for d, store, c0 in ((0, wq_ts, 0), (1, wk_ts, C)):
                    t = wqk.tile([128, 8, 128], F32, tag=f"w{d}",
                                 name=f"w{d}_{hp}")
                    nc.sync.dma_start(
                        out=f32r(t),
                        in_=f32r(wqkv_h[:, c0 + hp * 128:c0 + (hp + 1) * 128]
                                 .rearrange("(a p) c -> p a c", p=128)))
                    store[hp] = t

            fetch_wqk(0)
            nc.sync.dma_start(out=f32r(pshuf_sb), in_=f32r(pshuf_h[:, :]))
            nc.sync.dma_start(out=cos_sb, in_=cos_h[:, :])
            nc.sync.dma_start(out=sin_sb, in_=sin_h[:, :])
            fetch_wqk(1)
            fetch_wqk(2)

            # ---------------- QKV / V emission units ----------------
            def qk_units(hp):
                """Filler units for head-pair hp: per d in (Q, K), per qc
                half: 2 matmul units into a 1-bank PSUM tile, then an
                evac+shuffle+rotate unit."""
                units = []
                for d, wst, dst in ((0, wq_ts, qrot), (1, wk_ts, krot)):
                    cell = {}

                    def mm_u(qc, lo, d=d, wst=wst, cell=cell, hp=hp):
                        if lo == 0:
                            cell[qc] = pqk.tile([128, 512], F32, tag="qk",
                                                name=f"qk{d}_{hp}_{qc}")
                        ps = cell[qc]
                        for kb in range(lo, lo + 4):
                            MM(ps, f32r(wst[hp][:, kb, :]),
                               f32r(xT[:, kb, qc * 512:(qc + 1) * 512]),
                               start=(kb == 0), stop=(kb == 7))

                    def rot_u(qc, d=d, dst=dst, cell=cell, hp=hp):
                        if qc == 0:
                            cell["q"] = qsbp.tile([128, N], F32, tag="q",
                                                  name=f"q{d}_{hp}")
                        q_sb = cell["q"]
                        h0, h1 = qc * 512, (qc + 1) * 512
                        nc.vector.tensor_copy(f32r(q_sb[:, h0:h1]), cell[qc])
                        sh = psh.tile([128, 512], F32, tag="sh",
                                      name=f"sh{d}_{hp}_{qc}")
                        MM(sh, f32r(pshuf_sb), f32r(q_sb[:, h0:h1]),
                           start=True, stop=True)
                        tm = tmpp.tile([128, 512], F32, tag="t",
                                       name=f"t{d}_{hp}_{qc}")
                        nc.vector.tensor_mul(tm, sh, sin_sb[:, h0:h1])
                        nc.vector.tensor_mul(dst[:, hp, h0:h1],
                                             q_sb[:, h0:h1], cos_sb[:, h0:h1])
                        nc.vector.tensor_add(dst[:, hp, h0:h1],
                                             dst[:, hp, h0:h1], tm)

                    for qc in range(2):
                        units.append(lambda qc=qc, f=mm_u: f(qc, 0))
                        units.append(lambda qc=qc, f=mm_u: f(qc, 4))
                        units.append(lambda qc=qc, f=rot_u: f(qc))
                if hp + 3 < 8:
                    units.append(lambda hp=hp: fetch_wqk(hp + 3))
                return units

            def v_units(og, tb_pair):
                """V for tok-blocks (2*tb_pair, 2*tb_pair+1), heads og*8..+8,
                accumulated in a [128,1024] tile from the lg pool."""
                units = []
                cell = {}

                def mm_u(t, lo, og=og, tb_pair=tb_pair, cell=cell):
                    if t == 0 and lo == 0:
                        cell["ps"] = plg.tile([128, N], F32, tag="lg",
                                              name=f"v{og}_{tb_pair}")
                    ps = cell["ps"]
                    tb = 2 * tb_pair + t
                    for kb in range(lo, lo + 4):
                        MM(ps[:, t * 512:(t + 1) * 512],
                           f32r(xT[:, kb, tb * 128:(tb + 1) * 128]),
                           f32r(wv_cell[og][:, kb, :]),
                           start=(kb == 0), stop=(kb == 7))

                def evac_u(og=og, tb_pair=tb_pair, cell=cell):
                    nc.scalar.copy(
                        vext[:, 2 * tb_pair:2 * tb_pair + 2,
                             og * 8:(og + 1) * 8, 0:64],
                        cell["ps"].rearrange("p (a b c) -> p a b c", a=2, b=8))

                for t in range(2):
                    units.append(lambda t=t, f=mm_u: f(t, 0))
                    units.append(lambda t=t, f=mm_u: f(t, 4))
                units.append(evac_u)
                return units

            # ---------------- attention emission ----------------
            def lg_unit(h, kt, estore):
                hp, r0 = h // 2, (h % 2) * 64
                lg_t = plg.tile([128, N], F32, tag="lg", name=f"lg{h}_{kt}")
                for qc in range(2):
                    MM(lg_t[:, qc * 512:(qc + 1) * 512],
                       krot[r0:r0 + 64, hp, kt * 128:(kt + 1) * 128],
                       qrot[r0:r0 + 64, hp, qc * 512:(qc + 1) * 512],
                       start=True, stop=True)
                e_t = ep.tile([128, N], BF16, tag="e", name=f"e{h}_{kt}")
                nc.scalar.activation(e_t, lg_t, EXP, scale=0.125)
                estore[kt] = e_t

            def av_unit(h, kt, estore, av_ts):
                # start=True zeroes the whole PSUM bank: only the first group
                # per bank (qb 0 / 4) may use it; later groups accumulate
                # onto the freshly-zeroed bank.
                e_t = estore[kt]
                for qb in range(8):
                    t = av_ts[qb // 4]
                    j = qb % 4
                    MM(t[:, j * 128:j * 128 + 65],
                       e_t[:, qb * 128:(qb + 1) * 128],
                       vext[:, kt, h, :],
                       start=(kt == 0 and j == 0), stop=(kt == 7),
                       skip_group_check=True)

            def norm_unit(h, av_ts):
                hp, cr0 = h // 2, (h % 2) * 64
                for half in range(2):
                    t = av_ts[half].rearrange("p (a b) -> p a b", a=4)
                    nc.vector.reciprocal(
                        recip[:, h, half * 4:half * 4 + 4], t[:, :, 64])
                for qb in range(8):
                    t = av_ts[qb // 4].rearrange("p (a b) -> p a b", a=4)
                    nc.vector.tensor_scalar(
                        atok[:, hp, qb, cr0:cr0 + 64],
                        t[:, qb % 4, 0:64],
                        recip[:, h, qb:qb + 1], None, MULT)
                if h % 2 == 1:
                    hp = h // 2
                    # tracked guard read: the XBAR transpose's input deps are
                    # not honored; an ordinary DMA on the same SP queue makes
                    # the sequencer wait for the normalize writes first.
                    nc.sync.dma_start(out=guard_h[:, hp * 8:(hp + 1) * 8],
                                      in_=atok[:, hp, 7, 120:128])
                    nc.sync.dma_start(out=attn_Ts[hp],
                                      in_=atok[:, hp, :, :], transpose=True)

            def head_steps(h):
                """Returns (lg_steps, av_steps): 8 of each; av(kt==7) also
                emits the normalize (+ transpose for odd heads)."""
                state = {"e": {}}

                def lg_step(kt, h=h, state=state):
                    lg_unit(h, kt, state["e"])

                def av_step(kt, h=h, state=state):
                    if kt == 0:
                        state["av"] = [
                            pav.tile([128, 512], F32, tag="av",
                                     name=f"av{h}_{i}") for i in range(2)]
                    av_unit(h, kt, state["e"], state["av"])
                    if kt == 7:
                        norm_unit(h, state["av"])

                return ([lambda kt=kt, f=lg_step: f(kt) for kt in range(8)],
                        [lambda kt=kt, f=av_step: f(kt) for kt in range(8)])

            # ---------------- seg0: V(og0) + Q/K(hp0), no attention -------
            s0 = deque()
            for tb_pair in range(4):
                s0.extend(v_units(0, tb_pair))
            qk0 = deque(qk_units(0))
            while s0 or qk0:
                for _ in range(2):
                    if s0:
                        s0.popleft()()
                if qk0:
                    qk0.popleft()()

            # ---------------- segs 1..8: attention + fillers --------------
            lagq = deque()      # av steps lagging 2 behind the lg stream
            fillers = deque()
            fetch_wv(1)
            for seg in range(1, 9):
                if seg < 8:
                    fillers.extend(qk_units(seg))
                if seg <= 4:
                    fillers.extend(v_units(1, seg - 1))
                steps_left = 16
                for h in (2 * (seg - 1), 2 * (seg - 1) + 1):
                    lg_steps, av_steps = head_steps(h)
                    for kt in range(8):
                        lg_steps[kt]()
                        lagq.append(av_steps[kt])
                        if len(lagq) > 2:
                            lagq.popleft()()
                        # pace fillers evenly across the segment
                        budget = 2 if len(fillers) > steps_left else 1
                        for _ in range(budget):
                            if fillers:
                                fillers.popleft()()
                        steps_left -= 1
            while lagq:
                lagq.popleft()()

        # ---------------- proj + bias + output ----------------
        with tc.tile_pool(name="wpp", bufs=1) as wpp, \
             tc.tile_pool(name="ysb", bufs=2) as ysbp, \
             tc.tile_pool(name="psy", bufs=2, space="PSUM") as psy:
            wp_sb = wpp.tile([128, 8, C], BF16)
            for cb in range(8):
                nc.sync.dma_start(
                    out=wp_sb[:, cb, :],
                    in_=wpb_h[cb * 128:(cb + 1) * 128, :])
            nc.sync.dma_start(out=bb_sb, in_=bb_h[:, :])

            for tb in range(8):
                y_ps = psy.tile([128, C], F32, tag="y", name=f"y{tb}")
                for cb in range(8):
                    for oc in range(2):
                        MM(y_ps[:, oc * 512:(oc + 1) * 512],
                           attn_Ts[cb][:, tb, :],
                           wp_sb[:, cb, oc * 512:(oc + 1) * 512],
                           start=(cb == 0), stop=(cb == 7))
                y_sb = ysbp.tile([128, C], F32, tag="ysb", name=f"ysb{tb}")
                nc.vector.tensor_add(y_sb, y_ps, bb_sb)
                nc.sync.dma_start(out=out_h[tb * 128:(tb + 1) * 128, :],
                                  in_=y_sb)

    nc.finalize()
    return nc


_PROGRAM = None


def kernel(x, w_qkv, w_proj, b_proj):
    global _PROGRAM
    if _PROGRAM is None:
        _PROGRAM = _build_program()
    nc = _PROGRAM

    import ml_dtypes

    cos_d, sin_d, pshuf = _host_tables()
    shared = {
        "w_qkv": np.ascontiguousarray(w_qkv, np.float32),
        "w_projb": np.ascontiguousarray(
            np.asarray(w_proj, np.float32).astype(ml_dtypes.bfloat16)
        ).view(np.uint16),
        "b_bcast": np.ascontiguousarray(
            np.broadcast_to(np.asarray(b_proj, np.float32).reshape(1, C),
                            (128, C))),
        "cos_d": cos_d,
        "sin_d": sin_d,
        "pshuf": pshuf,
    }
    in_maps = [
        {"xt": np.ascontiguousarray(np.asarray(x[b], np.float32).T), **shared}
        for b in range(NCORES)
    ]
    res = run_bass_kernel_spmd(nc, in_maps, core_ids=list(range(NCORES)))
    return np.stack([res.results[b]["out"] for b in range(NCORES)], axis=0)


if __name__ == "__main__":
    xs = np.random.randn(B, N, C).astype(np.float32)
    wq = (np.random.randn(C, 3 * C) / np.sqrt(C)).astype(np.float32)
    wp = (np.random.randn(C, C) / np.sqrt(C)).astype(np.float32)
    bp = (np.random.randn(C) * 0.01).astype(np.float32)
    out = kernel(x=xs, w_qkv=wq, w_proj=wp, b_proj=bp)
    print(out.shape, out.dtype)


# revision 19
# speedup vs baseline: 1.4361x; 1.0433x over previous
"""Trainium2 Bass kernel for nn_Attention_5514738008849.

Dense transformer attention block with axial rotary embeddings:
  x:(8,1024,1024) -> qkv -> rope(q,k) -> softmax(qk^T/sqrt(d)) v -> proj+bias

Sharding: pure data-parallel over batch B=8 across the 8 NeuronCores (one
batch element per core, full weights replicated). No collectives.

Per-core dataflow, software-pipelined so the PE stream never drains (the PE
executes its stream in order, so emission order IS the schedule):
  - seg0: V(heads 0-7) + Q/K(head-pair 0); V(heads 8-15) spread over segs 1-4
  - segs 1..8: attention for the heads of hp-1, with QKV+rotary work for
    head-pair hp interleaved between attention steps as PE filler
  - attention per head: logits(kt) on a double-buffered [128,1024] PSUM pair;
    exp on ACT (scale=1/8) -> bf16; the AV matmuls lag the logits stream by
    two steps so they never wait on an exp in flight
  - AV transposed: stationary = exp tile [128k,128q], moving = V|ones bf16
    [128k,65] => out[q-block, d|rowsum]; 65 PE cycles/matmul instead of 512
    (only the first accumulation group per PSUM bank may use start=True:
    start zeroes the whole bank)
  - softmax normalize fused into the PSUM evacuation: DVE tensor_scalar by
    the per-(q,qb) reciprocal row-sum, writing bf16 token-major attn
  - attn transposed to c-major via the DMA XBAR (bf16, off the compute
    engines) per head-pair; a tracked guard DMA on the same queue forces the
    XBAR (whose input deps aren't honored) to wait for the normalize
  - proj: bf16 attn^T x bf16 w_proj (host-cast); bias added on the DVE
    evacuation via a host-replicated [128,1024] bias tile; fp32 output
"""

import os
import sys

sys.path.insert(0, "/opt/trn_rl_repo")

# This kernel needs the axon-tunneled NeuronCores. A JAX_PLATFORMS=cpu pin
# (used by some harnesses for the jax reference) would prevent the axon
# backend from registering; clearing it here is a no-op when jax has already
# initialized and restores device visibility when it hasn't.
if os.environ.get("JAX_PLATFORMS", "") not in ("", None):
    if "axon" not in os.environ["JAX_PLATFORMS"]:
        os.environ.pop("JAX_PLATFORMS", None)

from collections import deque

import numpy as np

import concourse.bass as bass
import concourse.bacc as bacc_mod
import concourse.mybir as mybir
from concourse.bass_utils import run_bass_kernel_spmd
from concourse.tile import TileContext

B, N, C = 8, 1024, 1024
H, D = 16, 64          # heads, head dim
ROT = 32               # rotary dims per head (head_dim // 2)
FH = FW = 32           # token grid for axial rope
NCORES = 8
F32 = mybir.dt.float32
F32R = mybir.dt.float32r
BF16 = mybir.dt.bfloat16


def _host_tables():
    """Rotary cos/sin in d-major (dim-on-partition) layout + shuffle matrix."""
    dim_r = D // 4                                    # 16
    base = np.linspace(1.0, (FH * FW) / 2.0, dim_r // 2) * np.pi   # (8,)

    def axis_freqs(n):
        pos = np.linspace(-1.0, 1.0, n)
        f = pos[:, None] * base[None, :]              # (n, 8)
        return np.repeat(f, 2, axis=-1)               # (n, 16)

    fH = np.broadcast_to(axis_freqs(FH)[:, None, :], (FH, FW, dim_r))
    fW = np.broadcast_to(axis_freqs(FW)[None, :, :], (FH, FW, dim_r))
    freqs = np.concatenate([fH, fW], axis=-1).reshape(N, ROT)      # (1024, 32)

    # d-major table for one 128-partition block = two heads:
    # rows 0-31 rot (head even), 32-63 pass, 64-95 rot (head odd), 96-127 pass
    cos_d = np.ones((128, N), np.float32)
    sin_d = np.zeros((128, N), np.float32)
    ct = np.cos(freqs).T.astype(np.float32)           # (32, 1024)
    st = np.sin(freqs).T.astype(np.float32)
    cos_d[0:32] = ct
    cos_d[64:96] = ct
    sin_d[0:32] = st
    sin_d[64:96] = st

    # signed permutation: shuf[2i] = -q[2i+1], shuf[2i+1] = q[2i] on rot rows
    pshuf = np.zeros((128, 128), np.float32)
    for off in (0, 64):
        for i in range(ROT // 2):
            r0, r1 = off + 2 * i, off + 2 * i + 1
            pshuf[r1, r0] = -1.0                      # out[r0] = -in[r1]
            pshuf[r0, r1] = 1.0                       # out[r1] = +in[r0]

    return cos_d, sin_d, pshuf


def _build_program():
    nc = bacc_mod.Bacc()
    xt_h = nc.declare_dram_parameter("xt", [C, N], F32, isOutput=False)
    wqkv_h = nc.declare_dram_parameter("w_qkv", [C, 3 * C], F32, isOutput=False)
    wpb_h = nc.declare_dram_parameter("w_projb", [C, C], BF16, isOutput=False)
    bb_h = nc.declare_dram_parameter("b_bcast", [128, C], F32, isOutput=False)
    cos_h = nc.declare_dram_parameter("cos_d", [128, N], F32, isOutput=False)
    sin_h = nc.declare_dram_parameter("sin_d", [128, N], F32, isOutput=False)
    pshuf_h = nc.declare_dram_parameter("pshuf", [128, 128], F32, isOutput=False)
    out_h = nc.declare_dram_parameter("out", [N, C], F32, isOutput=True)
    guard_h = nc.declare_dram_parameter("guard", [128, 64], BF16, isOutput=True)

    def f32r(ap):
        return ap.bitcast(F32R)

    MM = nc.tensor.matmul
    EXP = mybir.ActivationFunctionType.Exp
    MULT = mybir.AluOpType.mult

    with nc.allow_low_precision(reason="fp32r/bf16 operands"), \
         TileContext(nc) as tc, \
         tc.tile_pool(name="consts", bufs=1) as consts, \
         tc.tile_pool(name="persist", bufs=1) as persist, \
         tc.tile_pool(name="wpp", bufs=1) as wpp:

        cos_sb = consts.tile([128, N], F32)
        sin_sb = consts.tile([128, N], F32)
        pshuf_sb = consts.tile([128, 128], F32)
        bb_sb = consts.tile([128, C], F32)

        qrot = persist.tile([128, 8, N], BF16)     # Q_rot^T  (d-major, bf16)
        krot = persist.tile([128, 8, N], BF16)
        vext = persist.tile([128, 8, H, 65], BF16)  # V | ones per tok-block
        atok = persist.tile([128, 8, 8, 128], BF16)  # [q, hp, qb, c-in-hp]
        # one whole tile per head-pair: the XBAR transpose requires an
        # offset-0 (whole-tile) output AP
        attn_Ts = [persist.tile([128, 8, 128], BF16, name=f"attn_T{i}")
                   for i in range(8)]
        recip = persist.tile([128, H, 8], F32)       # 1/rowsum per (q, h, qb)
        wp_sb = wpp.tile([128, 8, C], BF16)          # w_proj rows, host-cast

        # ones column of vext (softmax denominator accumulator)
        nc.gpsimd.memset(vext[:, :, :, 64:65], 1.0)

        with tc.tile_pool(name="ep", bufs=4) as ep, \
             tc.tile_pool(name="plg", bufs=2, space="PSUM") as plg, \
             tc.tile_pool(name="pav", bufs=2, space="PSUM") as pav, \
             tc.tile_pool(name="xtp", bufs=1) as xtp, \
             tc.tile_pool(name="wqk", bufs=3) as wqk, \
             tc.tile_pool(name="wvp", bufs=1) as wvp, \
             tc.tile_pool(name="qsb", bufs=2) as qsbp, \
             tc.tile_pool(name="tmp", bufs=2) as tmpp, \
             tc.tile_pool(name="pqk", bufs=1, space="PSUM") as pqk, \
             tc.tile_pool(name="psh", bufs=1, space="PSUM") as psh:

            # ---------------- DMA issue (emission order = queue order) ----
            wv_cell = {}

            def fetch_wv(og):
                t = wvp.tile([128, 8, 512], F32, tag="wv", name=f"wv{og}")
                nc.gpsimd.dma_start(
                    out=f32r(t),
                    in_=f32r(wqkv_h[:, 2048 + og * 512:2048 + (og + 1) * 512]
                             .rearrange("(a p) c -> p a c", p=128)))
                wv_cell[og] = t

            # x^T first in kb halves (the first Q/K matmuls only need kb 0);
            # the V weights follow kb3 so seg0's V work lands mid-segment
            xT = xtp.tile([128, 8, N], F32)

            def fetch_xt(kb):
                nc.gpsimd.dma_start(
                    out=f32r(xT[:, kb, :]),
                    in_=f32r(xt_h[kb * 128:(kb + 1) * 128, :]))

            for kb in range(4):
                fetch_xt(kb)
            fetch_wv(0)
            for kb in range(4, 8):
                fetch_xt(kb)

            wq_ts, wk_ts = {}, {}

            def fetch_wqk(hp):
                for d, store, c0 in ((0, wq_ts, 0), (1, wk_ts, C)):
                    t = wqk.tile([128, 8, 128], F32, tag=f"w{d}",
                                 name=f"w{d}_{hp}")
                    nc.sync.dma_start(
                        out=f32r(t),
                        in_=f32r(wqkv_h[:, c0 + hp * 128:c0 + (hp + 1) * 128]
                                 .rearrange("(a p) c -> p a c", p=128)))
                    store[hp] = t

            fetch_wqk(0)
            nc.sync.dma_start(out=f32r(pshuf_sb), in_=f32r(pshuf_h[:, :]))
            nc.sync.dma_start(out=cos_sb, in_=cos_h[:, :])
            nc.sync.dma_start(out=sin_sb, in_=sin_h[:, :])
            fetch_wqk(1)
            fetch_wqk(2)

            # ---------------- QKV / V emission units ----------------
            def qk_units(hp):
                """Filler units for head-pair hp: per d in (Q, K), per qc
                half: 2 matmul units into a 1-bank PSUM tile, then an
                evac+shuffle+rotate unit."""
                units = []
                for d, wst, dst in ((0, wq_ts, qrot), (1, wk_ts, krot)):
                    cell = {}

                    def mm_u(qc, lo, d=d, wst=wst, cell=cell, hp=hp):
                        if lo == 0:
                            cell[qc] = pqk.tile([128, 512], F32, tag="qk",
                                                name=f"qk{d}_{hp}_{qc}")
                        ps = cell[qc]
                        for kb in range(lo, lo + 2):
                            MM(ps, f32r(wst[hp][:, kb, :]),
                               f32r(xT[:, kb, qc * 512:(qc + 1) * 512]),
                               start=(kb == 0), stop=(kb == 7))

                    def rot_u(qc, d=d, dst=dst, cell=cell, hp=hp):
                        if qc == 0:
                            cell["q"] = qsbp.tile([128, N], F32, tag="q",
                                                  name=f"q{d}_{hp}")
                        q_sb = cell["q"]
                        h0, h1 = qc * 512, (qc + 1) * 512
                        nc.vector.tensor_copy(f32r(q_sb[:, h0:h1]), cell[qc])
                        sh = psh.tile([128, 512], F32, tag="sh",
                                      name=f"sh{d}_{hp}_{qc}")
                        MM(sh, f32r(pshuf_sb), f32r(q_sb[:, h0:h1]),
                           start=True, stop=True)
                        tm = tmpp.tile([128, 512], F32, tag="t",
                                       name=f"t{d}_{hp}_{qc}")
                        nc.vector.tensor_mul(tm, sh, sin_sb[:, h0:h1])
                        nc.vector.tensor_mul(dst[:, hp, h0:h1],
                                             q_sb[:, h0:h1], cos_sb[:, h0:h1])
                        nc.vector.tensor_add(dst[:, hp, h0:h1],
                                             dst[:, hp, h0:h1], tm)

                    for qc in range(2):
                        for lo in range(0, 8, 2):
                            units.append(lambda qc=qc, lo=lo, f=mm_u: f(qc, lo))
                        units.append(lambda qc=qc, f=rot_u: f(qc))
                if hp + 3 < 8:
                    units.append(lambda hp=hp: fetch_wqk(hp + 3))
                return units

            def v_units(og, tb_pair):
                """V for tok-blocks (2*tb_pair, 2*tb_pair+1), heads og*8..+8,
                accumulated in a [128,1024] tile from the lg pool."""
                units = []
                cell = {}

                def mm_u(t, lo, og=og, tb_pair=tb_pair, cell=cell):
                    if t == 0 and lo == 0:
                        cell["ps"] = plg.tile([128, N], F32, tag="lg",
                                              name=f"v{og}_{tb_pair}")
                    ps = cell["ps"]
                    tb = 2 * tb_pair + t
                    for kb in range(lo, lo + 2):
                        MM(ps[:, t * 512:(t + 1) * 512],
                           f32r(xT[:, kb, tb * 128:(tb + 1) * 128]),
                           f32r(wv_cell[og][:, kb, :]),
                           start=(kb == 0), stop=(kb == 7))

                def evac_u(og=og, tb_pair=tb_pair, cell=cell):
                    nc.scalar.copy(
                        vext[:, 2 * tb_pair:2 * tb_pair + 2,
                             og * 8:(og + 1) * 8, 0:64],
                        cell["ps"].rearrange("p (a b c) -> p a b c", a=2, b=8))

                for t in range(2):
                    for lo in range(0, 8, 2):
                        units.append(lambda t=t, lo=lo, f=mm_u: f(t, lo))
                units.append(evac_u)
                return units

            # ---------------- attention emission ----------------
            def lg_unit(h, kt, estore):
                hp, r0 = h // 2, (h % 2) * 64
                lg_t = plg.tile([128, N], F32, tag="lg", name=f"lg{h}_{kt}")
                for qc in range(2):
                    MM(lg_t[:, qc * 512:(qc + 1) * 512],
                       krot[r0:r0 + 64, hp, kt * 128:(kt + 1) * 128],
                       qrot[r0:r0 + 64, hp, qc * 512:(qc + 1) * 512],
                       start=True, stop=True)
                e_t = ep.tile([128, N], BF16, tag="e", name=f"e{h}_{kt}")
                nc.scalar.activation(e_t, lg_t, EXP, scale=0.125)
                estore[kt] = e_t

            def av_unit(h, kt, estore, av_ts):
                # start=True zeroes the whole PSUM bank: only the first group
                # per bank (qb 0 / 4) may use it; later groups accumulate
                # onto the freshly-zeroed bank.
                e_t = estore[kt]
                for qb in range(8):
                    t = av_ts[qb // 4]
                    j = qb % 4
                    MM(t[:, j * 128:j * 128 + 65],
                       e_t[:, qb * 128:(qb + 1) * 128],
                       vext[:, kt, h, :],
                       start=(kt == 0 and j == 0), stop=(kt == 7),
                       skip_group_check=True)

            def norm_unit(h, av_ts):
                hp, cr0 = h // 2, (h % 2) * 64
                for half in range(2):
                    t = av_ts[half].rearrange("p (a b) -> p a b", a=4)
                    nc.vector.reciprocal(
                        recip[:, h, half * 4:half * 4 + 4], t[:, :, 64])
                for qb in range(8):
                    t = av_ts[qb // 4].rearrange("p (a b) -> p a b", a=4)
                    nc.vector.tensor_scalar(
                        atok[:, hp, qb, cr0:cr0 + 64],
                        t[:, qb % 4, 0:64],
                        recip[:, h, qb:qb + 1], None, MULT)
                if h % 2 == 1:
                    hp = h // 2
                    # tracked guard read: the XBAR transpose's input deps are
                    # not honored; an ordinary DMA on the same SP queue makes
                    # the sequencer wait for the normalize writes first.
                    nc.sync.dma_start(out=guard_h[:, hp * 8:(hp + 1) * 8],
                                      in_=atok[:, hp, 7, 120:128])
                    nc.sync.dma_start(out=attn_Ts[hp],
                                      in_=atok[:, hp, :, :], transpose=True)

            def head_steps(h):
                """Returns (lg_steps, av_steps): 8 of each; av(kt==7) also
                emits the normalize (+ transpose for odd heads)."""
                state = {"e": {}}

                def lg_step(kt, h=h, state=state):
                    lg_unit(h, kt, state["e"])

                def av_step(kt, h=h, state=state):
                    if kt == 0:
                        state["av"] = [
                            pav.tile([128, 512], F32, tag="av",
                                     name=f"av{h}_{i}") for i in range(2)]
                    av_unit(h, kt, state["e"], state["av"])
                    if kt == 7:
                        norm_unit(h, state["av"])

                return ([lambda kt=kt, f=lg_step: f(kt) for kt in range(8)],
                        [lambda kt=kt, f=av_step: f(kt) for kt in range(8)])

            # ---------------- seg0: Q/K(hp0) leading, V(og0) behind -------
            s0 = deque()
            for tb_pair in range(4):
                s0.extend(v_units(0, tb_pair))
            qk0 = deque(qk_units(0))
            # qk first (x^T kb0 + wq0 arrive first); V follows as its
            # weights land mid-segment
            while s0 or qk0:
                if qk0:
                    qk0.popleft()()
                for _ in range(2):
                    if s0:
                        s0.popleft()()

            # ---------------- segs 1..8: attention + fillers --------------
            lagq = deque()      # av steps lagging 2 behind the lg stream
            fillers = deque()
            fetch_wv(1)
            for seg in range(1, 9):
                if seg < 8:
                    fillers.extend(qk_units(seg))
                if seg <= 4:
                    fillers.extend(v_units(1, seg - 1))
                if seg == 6:
                    # prefetch proj weights + bias on the now-quiet SP queue
                    for cb in range(8):
                        nc.sync.dma_start(
                            out=wp_sb[:, cb, :],
                            in_=wpb_h[cb * 128:(cb + 1) * 128, :])
                    nc.sync.dma_start(out=bb_sb, in_=bb_h[:, :])
                steps_left = 16
                for h in (2 * (seg - 1), 2 * (seg - 1) + 1):
                    lg_steps, av_steps = head_steps(h)
                    for kt in range(8):
                        lg_steps[kt]()
                        lagq.append(av_steps[kt])
                        if len(lagq) > 2:
                            lagq.popleft()()
                        # pace fillers evenly across the segment
                        budget = 2 if len(fillers) > steps_left else 1
                        for _ in range(budget):
                            if fillers:
                                fillers.popleft()()
                        steps_left -= 1
            while lagq:
                lagq.popleft()()

        # ---------------- proj + bias + output ----------------
        with tc.tile_pool(name="ysb", bufs=2) as ysbp, \
             tc.tile_pool(name="psy", bufs=2, space="PSUM") as psy:
            for tb in range(8):
                y_ps = psy.tile([128, C], F32, tag="y", name=f"y{tb}")
                for cb in range(8):
                    for oc in range(2):
                        MM(y_ps[:, oc * 512:(oc + 1) * 512],
                           attn_Ts[cb][:, tb, :],
                           wp_sb[:, cb, oc * 512:(oc + 1) * 512],
                           start=(cb == 0), stop=(cb == 7))
                y_sb = ysbp.tile([128, C], F32, tag="ysb", name=f"ysb{tb}")
                nc.vector.tensor_add(y_sb, y_ps, bb_sb)
                nc.sync.dma_start(out=out_h[tb * 128:(tb + 1) * 128, :],
                                  in_=y_sb)

    nc.finalize()
    return nc


_PROGRAM = None


def kernel(x, w_qkv, w_proj, b_proj):
    global _PROGRAM
    if _PROGRAM is None:
        _PROGRAM = _build_program()
    nc = _PROGRAM

    import ml_dtypes

    cos_d, sin_d, pshuf = _host_tables()
    shared = {
        "w_qkv": np.ascontiguousarray(w_qkv, np.float32),
        "w_projb": np.ascontiguousarray(
            np.asarray(w_proj, np.float32).astype(ml_dtypes.bfloat16)
        ).view(np.uint16),
        "b_bcast": np.ascontiguousarray(
            np.broadcast_to(np.asarray(b_proj, np.float32).reshape(1, C),
                            (128, C))),
        "cos_d": cos_d,
        "sin_d": sin_d,
        "pshuf": pshuf,
    }
    in_maps = [
        {"xt": np.ascontiguousarray(np.asarray(x[b], np.float32).T), **shared}
        for b in range(NCORES)
    ]
    res = run_bass_kernel_spmd(nc, in_maps, core_ids=list(range(NCORES)))
    return np.stack([res.results[b]["out"] for b in range(NCORES)], axis=0)


if __name__ == "__main__":
    xs = np.random.randn(B, N, C).astype(np.float32)
    wq = (np.random.randn(C, 3 * C) / np.sqrt(C)).astype(np.float32)
    wp = (np.random.randn(C, C) / np.sqrt(C)).astype(np.float32)
    bp = (np.random.randn(C) * 0.01).astype(np.float32)
    out = kernel(x=xs, w_qkv=wq, w_proj=wp, b_proj=bp)
    print(out.shape, out.dtype)


# revision 26
# speedup vs baseline: 1.4538x; 1.0124x over previous
"""Trainium2 Bass kernel for nn_Attention_5514738008849.

Dense transformer attention block with axial rotary embeddings:
  x:(8,1024,1024) -> qkv -> rope(q,k) -> softmax(qk^T/sqrt(d)) v -> proj+bias

Sharding: pure data-parallel over batch B=8 across the 8 NeuronCores (one
batch element per core, full weights replicated). No collectives.

Per-core dataflow, software-pipelined so the PE stream never drains (the PE
executes its stream in order, so emission order IS the schedule):
  - seg0: V(heads 0-7) + Q/K(head-pair 0); V(heads 8-15) spread over segs 1-4
  - segs 1..8: attention for the heads of hp-1, with QKV+rotary work for
    head-pair hp interleaved between attention steps as PE filler
  - attention per head: logits(kt) on a double-buffered [128,1024] PSUM pair;
    exp on ACT (scale=1/8) -> bf16; the AV matmuls lag the logits stream by
    two steps so they never wait on an exp in flight
  - AV transposed: stationary = exp tile [128k,128q], moving = V|ones bf16
    [128k,65] => out[q-block, d|rowsum]; 65 PE cycles/matmul instead of 512
    (only the first accumulation group per PSUM bank may use start=True:
    start zeroes the whole bank)
  - softmax normalize fused into the PSUM evacuation: DVE tensor_scalar by
    the per-(q,qb) reciprocal row-sum, writing bf16 token-major attn
  - attn transposed to c-major via the DMA XBAR (bf16, off the compute
    engines) per head-pair; a tracked guard DMA on the same queue forces the
    XBAR (whose input deps aren't honored) to wait for the normalize
  - proj: bf16 attn^T x bf16 w_proj (host-cast); bias added on the DVE
    evacuation via a host-replicated [128,1024] bias tile; fp32 output
"""

import os
import sys

sys.path.insert(0, "/opt/trn_rl_repo")

# This kernel needs the axon-tunneled NeuronCores. A JAX_PLATFORMS=cpu pin
# (used by some harnesses for the jax reference) would prevent the axon
# backend from registering; clearing it here is a no-op when jax has already
# initialized and restores device visibility when it hasn't.
if os.environ.get("JAX_PLATFORMS", "") not in ("", None):
    if "axon" not in os.environ["JAX_PLATFORMS"]:
        os.environ.pop("JAX_PLATFORMS", None)

from collections import deque

import numpy as np

import concourse.bass as bass
import concourse.bacc as bacc_mod
import concourse.mybir as mybir
from concourse.bass_utils import run_bass_kernel_spmd
from concourse.tile import TileContext

B, N, C = 8, 1024, 1024
H, D = 16, 64          # heads, head dim
ROT = 32               # rotary dims per head (head_dim // 2)
FH = FW = 32           # token grid for axial rope
NCORES = 8
F32 = mybir.dt.float32
F32R = mybir.dt.float32r
BF16 = mybir.dt.bfloat16


def _host_tables():
    """Rotary cos/sin in d-major (dim-on-partition) layout + shuffle matrix."""
    dim_r = D // 4                                    # 16
    base = np.linspace(1.0, (FH * FW) / 2.0, dim_r // 2) * np.pi   # (8,)

    def axis_freqs(n):
        pos = np.linspace(-1.0, 1.0, n)
        f = pos[:, None] * base[None, :]              # (n, 8)
        return np.repeat(f, 2, axis=-1)               # (n, 16)

    fH = np.broadcast_to(axis_freqs(FH)[:, None, :], (FH, FW, dim_r))
    fW = np.broadcast_to(axis_freqs(FW)[None, :, :], (FH, FW, dim_r))
    freqs = np.concatenate([fH, fW], axis=-1).reshape(N, ROT)      # (1024, 32)

    # d-major table for one 128-partition block = two heads:
    # rows 0-31 rot (head even), 32-63 pass, 64-95 rot (head odd), 96-127 pass
    cos_d = np.ones((128, N), np.float32)
    sin_d = np.zeros((128, N), np.float32)
    ct = np.cos(freqs).T.astype(np.float32)           # (32, 1024)
    st = np.sin(freqs).T.astype(np.float32)
    cos_d[0:32] = ct
    cos_d[64:96] = ct
    sin_d[0:32] = st
    sin_d[64:96] = st

    # signed permutation: shuf[2i] = -q[2i+1], shuf[2i+1] = q[2i] on rot rows
    pshuf = np.zeros((128, 128), np.float32)
    for off in (0, 64):
        for i in range(ROT // 2):
            r0, r1 = off + 2 * i, off + 2 * i + 1
            pshuf[r1, r0] = -1.0                      # out[r0] = -in[r1]
            pshuf[r0, r1] = 1.0                       # out[r1] = +in[r0]

    return cos_d, sin_d, pshuf


def _build_program():
    nc = bacc_mod.Bacc()
    xt_h = nc.declare_dram_parameter("xt", [C, N], F32, isOutput=False)
    wqkv_h = nc.declare_dram_parameter("w_qkv", [C, 3 * C], F32, isOutput=False)
    wpb_h = nc.declare_dram_parameter("w_projb", [C, C], BF16, isOutput=False)
    bb_h = nc.declare_dram_parameter("b_bcast", [128, C], F32, isOutput=False)
    cos_h = nc.declare_dram_parameter("cos_d", [128, N], F32, isOutput=False)
    sin_h = nc.declare_dram_parameter("sin_d", [128, N], F32, isOutput=False)
    pshuf_h = nc.declare_dram_parameter("pshuf", [128, 128], F32, isOutput=False)
    out_h = nc.declare_dram_parameter("out", [N, C], F32, isOutput=True)
    guard_h = nc.declare_dram_parameter("guard", [128, 64], BF16, isOutput=True)

    def f32r(ap):
        return ap.bitcast(F32R)

    MM = nc.tensor.matmul
    EXP = mybir.ActivationFunctionType.Exp
    MULT = mybir.AluOpType.mult

    with nc.allow_low_precision(reason="fp32r/bf16 operands"), \
         TileContext(nc) as tc, \
         tc.tile_pool(name="consts", bufs=1) as consts, \
         tc.tile_pool(name="persist", bufs=1) as persist, \
         tc.tile_pool(name="wpp", bufs=1) as wpp:

        cos_sb = consts.tile([128, N], F32)
        sin_sb = consts.tile([128, N], F32)
        pshuf_sb = consts.tile([128, 128], F32)
        bb_sb = consts.tile([128, C], F32)

        qrot = persist.tile([128, 8, N], BF16)     # Q_rot^T  (d-major, bf16)
        krot = persist.tile([128, 8, N], BF16)
        vext = persist.tile([128, 8, H, 65], BF16)  # V | ones per tok-block
        atok = persist.tile([128, 8, 8, 128], BF16)  # [q, hp, qb, c-in-hp]
        # one whole tile per head-pair: the XBAR transpose requires an
        # offset-0 (whole-tile) output AP
        attn_Ts = [persist.tile([128, 8, 128], BF16, name=f"attn_T{i}")
                   for i in range(8)]
        recip = persist.tile([128, H, 8], F32)       # 1/rowsum per (q, h, qb)
        wp_sb = wpp.tile([128, 8, C], BF16)          # w_proj rows, host-cast

        # ones column of vext (softmax denominator accumulator)
        nc.gpsimd.memset(vext[:, :, :, 64:65], 1.0)

        with tc.tile_pool(name="ep", bufs=4) as ep, \
             tc.tile_pool(name="plg", bufs=2, space="PSUM") as plg, \
             tc.tile_pool(name="pav", bufs=2, space="PSUM") as pav, \
             tc.tile_pool(name="xtp", bufs=1) as xtp, \
             tc.tile_pool(name="wqk", bufs=3) as wqk, \
             tc.tile_pool(name="wvp", bufs=1) as wvp, \
             tc.tile_pool(name="qsb", bufs=2) as qsbp, \
             tc.tile_pool(name="tmp", bufs=2) as tmpp, \
             tc.tile_pool(name="pqk", bufs=2, space="PSUM") as pqk:

            # ---------------- DMA issue (emission order = queue order) ----
            wv_cell = {}

            def fetch_wv(ch):
                # one chunk = 256 V columns = 4 heads
                t = wvp.tile([128, 8, 256], F32, tag="wv", name=f"wv{ch}")
                nc.gpsimd.dma_start(
                    out=f32r(t),
                    in_=f32r(wqkv_h[:, 2048 + ch * 256:2048 + (ch + 1) * 256]
                             .rearrange("(a p) c -> p a c", p=128)))
                wv_cell[ch] = t

            # x^T first in kb halves (the first Q/K matmuls only need kb 0);
            # the V weights follow kb3 so seg0's V work lands mid-segment
            xT = xtp.tile([128, 8, N], F32)

            def fetch_xt(kb):
                nc.gpsimd.dma_start(
                    out=f32r(xT[:, kb, :]),
                    in_=f32r(xt_h[kb * 128:(kb + 1) * 128, :]))

            for kb in range(4):
                fetch_xt(kb)
            fetch_wv(0)
            for kb in range(4, 8):
                fetch_xt(kb)

            wq_ts, wk_ts = {}, {}

            def fetch_wqk(hp):
                for d, store, c0 in ((0, wq_ts, 0), (1, wk_ts, C)):
                    t = wqk.tile([128, 8, 128], F32, tag=f"w{d}",
                                 name=f"w{d}_{hp}")
                    nc.sync.dma_start(
                        out=f32r(t),
                        in_=f32r(wqkv_h[:, c0 + hp * 128:c0 + (hp + 1) * 128]
                                 .rearrange("(a p) c -> p a c", p=128)))
                    store[hp] = t

            fetch_wqk(0)
            nc.sync.dma_start(out=f32r(pshuf_sb), in_=f32r(pshuf_h[:, :]))
            nc.sync.dma_start(out=cos_sb, in_=cos_h[:, :])
            nc.sync.dma_start(out=sin_sb, in_=sin_h[:, :])
            fetch_wqk(1)
            fetch_wqk(2)

            # ---------------- QKV / V emission units ----------------
            def qk_units(hp):
                """Filler units for head-pair hp: per d in (Q, K), per qc
                half: 2 matmul units into a 1-bank PSUM tile, then an
                evac+shuffle+rotate unit."""
                units = []
                for d, wst, dst in ((0, wq_ts, qrot), (1, wk_ts, krot)):
                    cell = {}

                    def mm_u(qc, lo, d=d, wst=wst, cell=cell, hp=hp):
                        if lo == 0:
                            cell[qc] = pqk.tile([128, 512], F32, tag="qk",
                                                name=f"qk{d}_{hp}_{qc}")
                        ps = cell[qc]
                        for kb in range(lo, lo + 2):
                            MM(ps, f32r(wst[hp][:, kb, :]),
                               f32r(xT[:, kb, qc * 512:(qc + 1) * 512]),
                               start=(kb == 0), stop=(kb == 7))

                    def rot_u(qc, d=d, dst=dst, cell=cell, hp=hp):
                        if qc == 0:
                            cell["q"] = qsbp.tile([128, N], F32, tag="q",
                                                  name=f"q{d}_{hp}")
                        q_sb = cell["q"]
                        h0, h1 = qc * 512, (qc + 1) * 512
                        nc.vector.tensor_copy(f32r(q_sb[:, h0:h1]), cell[qc])
                        sh = pqk.tile([128, 512], F32, tag="qk",
                                      name=f"sh{d}_{hp}_{qc}")
                        MM(sh, f32r(pshuf_sb), f32r(q_sb[:, h0:h1]),
                           start=True, stop=True)
                        tm = tmpp.tile([128, 512], F32, tag="t",
                                       name=f"t{d}_{hp}_{qc}")
                        nc.vector.tensor_mul(tm, sh, sin_sb[:, h0:h1])
                        nc.vector.tensor_mul(dst[:, hp, h0:h1],
                                             q_sb[:, h0:h1], cos_sb[:, h0:h1])
                        nc.vector.tensor_add(dst[:, hp, h0:h1],
                                             dst[:, hp, h0:h1], tm)

                    # kb-major across the two qc halves so compute tracks the
                    # x^T chunk arrivals
                    for lo in range(0, 8, 2):
                        for qc in range(2):
                            units.append(lambda qc=qc, lo=lo, f=mm_u: f(qc, lo))
                    units.append(lambda f=rot_u: f(0))
                    units.append(lambda f=rot_u: f(1))
                if hp + 3 < 8:
                    units.append(lambda hp=hp: fetch_wqk(hp + 3))
                return units

            def v_units(ch):
                """V chunk ch (256 cols = heads 4ch..4ch+3), all 8 tok-blocks,
                two [128,1024] lg-pool tiles of 4 tok-blocks each. Each tile's
                whole lifetime (create -> accumulate -> evac) is one atomic
                unit: the lg tag's buffer rotation cannot handle a tile whose
                accumulation interleaves with other tiles of the same tag.
                start=True only for the first region of each PSUM bank."""

                def lump(g, ch=ch):
                    ps = plg.tile([128, N], F32, tag="lg", name=f"v{ch}_{g}")
                    for lo in (0, 2, 4, 6):     # kb-major: tracks x^T arrival
                        for tb4 in range(4):
                            tb = g * 4 + tb4
                            for kb in (lo, lo + 1):
                                MM(ps[:, tb4 * 256:(tb4 + 1) * 256],
                                   f32r(xT[:, kb, tb * 128:(tb + 1) * 128]),
                                   f32r(wv_cell[ch][:, kb, :]),
                                   start=(kb == 0 and tb4 % 2 == 0),
                                   stop=(kb == 7), skip_group_check=True)
                    nc.scalar.copy(
                        vext[:, g * 4:(g + 1) * 4,
                             ch * 4:(ch + 1) * 4, 0:64],
                        ps.rearrange("p (a b c) -> p a b c", a=4, b=4))

                return [lambda g=g, f=lump: f(g) for g in range(2)]

            # ---------------- attention emission ----------------
            def lg_unit(h, kt, estore):
                hp, r0 = h // 2, (h % 2) * 64
                lg_t = plg.tile([128, N], F32, tag="lg", name=f"lg{h}_{kt}")
                for qc in range(2):
                    MM(lg_t[:, qc * 512:(qc + 1) * 512],
                       krot[r0:r0 + 64, hp, kt * 128:(kt + 1) * 128],
                       qrot[r0:r0 + 64, hp, qc * 512:(qc + 1) * 512],
                       start=True, stop=True)
                e_t = ep.tile([128, N], BF16, tag="e", name=f"e{h}_{kt}")
                nc.scalar.activation(e_t, lg_t, EXP, scale=0.125)
                estore[kt] = e_t

            def av_unit(h, kt, estore, av_ts):
                # start=True zeroes the whole PSUM bank: only the first group
                # per bank (qb 0 / 4) may use it; later groups accumulate
                # onto the freshly-zeroed bank.
                e_t = estore[kt]
                for qb in range(8):
                    t = av_ts[qb // 4]
                    j = qb % 4
                    MM(t[:, j * 128:j * 128 + 65],
                       e_t[:, qb * 128:(qb + 1) * 128],
                       vext[:, kt, h, :],
                       start=(kt == 0 and j == 0), stop=(kt == 7),
                       skip_group_check=True)

            def norm_unit(h, av_ts):
                hp, cr0 = h // 2, (h % 2) * 64
                for half in range(2):
                    t = av_ts[half].rearrange("p (a b) -> p a b", a=4)
                    nc.vector.reciprocal(
                        recip[:, h, half * 4:half * 4 + 4], t[:, :, 64])
                for qb in range(8):
                    t = av_ts[qb // 4].rearrange("p (a b) -> p a b", a=4)
                    nc.vector.tensor_scalar(
                        atok[:, hp, qb, cr0:cr0 + 64],
                        t[:, qb % 4, 0:64],
                        recip[:, h, qb:qb + 1], None, MULT)
                if h % 2 == 1:
                    hp = h // 2
                    # tracked guard read: the XBAR transpose's input deps are
                    # not honored; an ordinary DMA on the same SP queue makes
                    # the sequencer wait for the normalize writes first.
                    nc.sync.dma_start(out=guard_h[:, hp * 8:(hp + 1) * 8],
                                      in_=atok[:, hp, 7, 120:128])
                    nc.sync.dma_start(out=attn_Ts[hp],
                                      in_=atok[:, hp, :, :], transpose=True)

            def head_steps(h):
                """Returns (lg_steps, av_steps): 8 of each; av(kt==7) also
                emits the normalize (+ transpose for odd heads)."""
                state = {"e": {}}

                def lg_step(kt, h=h, state=state):
                    lg_unit(h, kt, state["e"])

                def av_step(kt, h=h, state=state):
                    if kt == 0:
                        state["av"] = [
                            pav.tile([128, 512], F32, tag="av",
                                     name=f"av{h}_{i}") for i in range(2)]
                    av_unit(h, kt, state["e"], state["av"])
                    if kt == 7:
                        norm_unit(h, state["av"])

                return ([lambda kt=kt, f=lg_step: f(kt) for kt in range(8)],
                        [lambda kt=kt, f=av_step: f(kt) for kt in range(8)])

            # ---------------- seg0: Q/K(hp0) leading, V chunk 0 behind ----
            s0 = deque(v_units(0))
            qk0 = deque(qk_units(0))
            # qk first (x^T kb0 + wq0 arrive first); V follows as its
            # weights land mid-segment
            while s0 or qk0:
                if qk0:
                    qk0.popleft()()
                if s0:
                    s0.popleft()()

            # ---------------- segs 1..8: attention + fillers --------------
            lagq = deque()      # av steps lagging 2 behind the lg stream
            fillers = deque()
            fetch_wv(1)
            for seg in range(1, 9):
                if seg < 8:
                    fillers.extend(qk_units(seg))
                if seg in (1, 2, 4):
                    # V chunks 1-3 (heads 4-15); chunk c needed first by
                    # attention head 4c (seg 2c+1)
                    fillers.extend(v_units(seg if seg < 4 else 3))
                    if seg < 4:
                        fetch_wv(seg + 1)
                if seg == 6:
                    # prefetch proj weights + bias on the now-quiet SP queue
                    for cb in range(8):
                        nc.sync.dma_start(
                            out=wp_sb[:, cb, :],
                            in_=wpb_h[cb * 128:(cb + 1) * 128, :])
                    nc.sync.dma_start(out=bb_sb, in_=bb_h[:, :])
                steps_left = 16
                for h in (2 * (seg - 1), 2 * (seg - 1) + 1):
                    lg_steps, av_steps = head_steps(h)
                    for kt in range(8):
                        lg_steps[kt]()
                        lagq.append(av_steps[kt])
                        if len(lagq) > 2:
                            lagq.popleft()()
                        # pace fillers evenly across the segment
                        budget = 2 if len(fillers) > steps_left else 1
                        for _ in range(budget):
                            if fillers:
                                fillers.popleft()()
                        steps_left -= 1
            while lagq:
                lagq.popleft()()

        # ---------------- proj + bias + output ----------------
        with tc.tile_pool(name="ysb", bufs=2) as ysbp, \
             tc.tile_pool(name="psy", bufs=2, space="PSUM") as psy:
            for tb in range(8):
                y_ps = psy.tile([128, C], F32, tag="y", name=f"y{tb}")
                for cb in range(8):
                    for oc in range(2):
                        MM(y_ps[:, oc * 512:(oc + 1) * 512],
                           attn_Ts[cb][:, tb, :],
                           wp_sb[:, cb, oc * 512:(oc + 1) * 512],
                           start=(cb == 0), stop=(cb == 7))
                y_sb = ysbp.tile([128, C], F32, tag="ysb", name=f"ysb{tb}")
                nc.vector.tensor_add(y_sb, y_ps, bb_sb)
                nc.sync.dma_start(out=out_h[tb * 128:(tb + 1) * 128, :],
                                  in_=y_sb)

    nc.finalize()
    return nc


_PROGRAM = None


def kernel(x, w_qkv, w_proj, b_proj):
    global _PROGRAM
    if _PROGRAM is None:
        _PROGRAM = _build_program()
    nc = _PROGRAM

    import ml_dtypes

    cos_d, sin_d, pshuf = _host_tables()
    shared = {
        "w_qkv": np.ascontiguousarray(w_qkv, np.float32),
        "w_projb": np.ascontiguousarray(
            np.asarray(w_proj, np.float32).astype(ml_dtypes.bfloat16)
        ).view(np.uint16),
        "b_bcast": np.ascontiguousarray(
            np.broadcast_to(np.asarray(b_proj, np.float32).reshape(1, C),
                            (128, C))),
        "cos_d": cos_d,
        "sin_d": sin_d,
        "pshuf": pshuf,
    }
    in_maps = [
        {"xt": np.ascontiguousarray(np.asarray(x[b], np.float32).T), **shared}
        for b in range(NCORES)
    ]
    res = run_bass_kernel_spmd(nc, in_maps, core_ids=list(range(NCORES)))
    return np.stack([res.results[b]["out"] for b in range(NCORES)], axis=0)


if __name__ == "__main__":
    xs = np.random.randn(B, N, C).astype(np.float32)
    wq = (np.random.randn(C, 3 * C) / np.sqrt(C)).astype(np.float32)
    wp = (np.random.randn(C, C) / np.sqrt(C)).astype(np.float32)
    bp = (np.random.randn(C) * 0.01).astype(np.float32)
    out = kernel(x=xs, w_qkv=wq, w_proj=wp, b_proj=bp)
    print(out.shape, out.dtype)


# revision 29
# speedup vs baseline: 1.4853x; 1.0216x over previous
"""Trainium2 Bass kernel for nn_Attention_5514738008849.

Dense transformer attention block with axial rotary embeddings:
  x:(8,1024,1024) -> qkv -> rope(q,k) -> softmax(qk^T/sqrt(d)) v -> proj+bias

Sharding: pure data-parallel over batch B=8 across the 8 NeuronCores (one
batch element per core, full weights replicated). No collectives.

Per-core dataflow, software-pipelined so the PE stream never drains (the PE
executes its stream in order, so emission order IS the schedule):
  - seg0: V(heads 0-7) + Q/K(head-pair 0); V(heads 8-15) spread over segs 1-4
  - segs 1..8: attention for the heads of hp-1, with QKV+rotary work for
    head-pair hp interleaved between attention steps as PE filler
  - attention per head: logits(kt) on a double-buffered [128,1024] PSUM pair;
    exp on ACT (scale=1/8) -> bf16; the AV matmuls lag the logits stream by
    two steps so they never wait on an exp in flight
  - AV transposed: stationary = exp tile [128k,128q], moving = V|ones bf16
    [128k,65] => out[q-block, d|rowsum]; 65 PE cycles/matmul instead of 512
    (only the first accumulation group per PSUM bank may use start=True:
    start zeroes the whole bank)
  - softmax normalize fused into the PSUM evacuation: DVE tensor_scalar by
    the per-(q,qb) reciprocal row-sum, writing bf16 token-major attn
  - attn transposed to c-major via the DMA XBAR (bf16, off the compute
    engines) per head-pair; a tracked guard DMA on the same queue forces the
    XBAR (whose input deps aren't honored) to wait for the normalize
  - proj: bf16 attn^T x bf16 w_proj (host-cast); bias added on the DVE
    evacuation via a host-replicated [128,1024] bias tile; fp32 output
"""

import os
import sys

sys.path.insert(0, "/opt/trn_rl_repo")

# This kernel needs the axon-tunneled NeuronCores. A JAX_PLATFORMS=cpu pin
# (used by some harnesses for the jax reference) would prevent the axon
# backend from registering; clearing it here is a no-op when jax has already
# initialized and restores device visibility when it hasn't.
if os.environ.get("JAX_PLATFORMS", "") not in ("", None):
    if "axon" not in os.environ["JAX_PLATFORMS"]:
        os.environ.pop("JAX_PLATFORMS", None)

from collections import deque

import numpy as np

import concourse.bass as bass
import concourse.bacc as bacc_mod
import concourse.mybir as mybir
from concourse.bass_utils import run_bass_kernel_spmd
from concourse.tile import TileContext

B, N, C = 8, 1024, 1024
H, D = 16, 64          # heads, head dim
ROT = 32               # rotary dims per head (head_dim // 2)
FH = FW = 32           # token grid for axial rope
NCORES = 8
F32 = mybir.dt.float32
F32R = mybir.dt.float32r
BF16 = mybir.dt.bfloat16


def _host_tables():
    """Rotary cos/sin in d-major (dim-on-partition) layout + shuffle matrix."""
    dim_r = D // 4                                    # 16
    base = np.linspace(1.0, (FH * FW) / 2.0, dim_r // 2) * np.pi   # (8,)

    def axis_freqs(n):
        pos = np.linspace(-1.0, 1.0, n)
        f = pos[:, None] * base[None, :]              # (n, 8)
        return np.repeat(f, 2, axis=-1)               # (n, 16)

    fH = np.broadcast_to(axis_freqs(FH)[:, None, :], (FH, FW, dim_r))
    fW = np.broadcast_to(axis_freqs(FW)[None, :, :], (FH, FW, dim_r))
    freqs = np.concatenate([fH, fW], axis=-1).reshape(N, ROT)      # (1024, 32)

    # d-major table for one 128-partition block = two heads:
    # rows 0-31 rot (head even), 32-63 pass, 64-95 rot (head odd), 96-127 pass
    cos_d = np.ones((128, N), np.float32)
    sin_d = np.zeros((128, N), np.float32)
    ct = np.cos(freqs).T.astype(np.float32)           # (32, 1024)
    st = np.sin(freqs).T.astype(np.float32)
    cos_d[0:32] = ct
    cos_d[64:96] = ct
    sin_d[0:32] = st
    sin_d[64:96] = st

    # signed permutation: shuf[2i] = -q[2i+1], shuf[2i+1] = q[2i] on rot rows
    pshuf = np.zeros((128, 128), np.float32)
    for off in (0, 64):
        for i in range(ROT // 2):
            r0, r1 = off + 2 * i, off + 2 * i + 1
            pshuf[r1, r0] = -1.0                      # out[r0] = -in[r1]
            pshuf[r0, r1] = 1.0                       # out[r1] = +in[r0]

    return cos_d, sin_d, pshuf


def _build_program():
    nc = bacc_mod.Bacc()
    xt_h = nc.declare_dram_parameter("xt", [C, N], F32, isOutput=False)
    wqkv_h = nc.declare_dram_parameter("w_qkv", [C, 3 * C], F32, isOutput=False)
    wpb_h = nc.declare_dram_parameter("w_projb", [C, C], BF16, isOutput=False)
    bb_h = nc.declare_dram_parameter("b_bcast", [128, C], F32, isOutput=False)
    cos_h = nc.declare_dram_parameter("cos_d", [128, N], F32, isOutput=False)
    sin_h = nc.declare_dram_parameter("sin_d", [128, N], F32, isOutput=False)
    pshuf_h = nc.declare_dram_parameter("pshuf", [128, 128], F32, isOutput=False)
    out_h = nc.declare_dram_parameter("out", [N, C], F32, isOutput=True)
    guard_h = nc.declare_dram_parameter("guard", [128, 64], BF16, isOutput=True)

    def f32r(ap):
        return ap.bitcast(F32R)

    MM = nc.tensor.matmul
    EXP = mybir.ActivationFunctionType.Exp
    MULT = mybir.AluOpType.mult

    with nc.allow_low_precision(reason="fp32r/bf16 operands"), \
         TileContext(nc) as tc, \
         tc.tile_pool(name="consts", bufs=1) as consts, \
         tc.tile_pool(name="persist", bufs=1) as persist, \
         tc.tile_pool(name="wpp", bufs=1) as wpp:

        cos_sb = consts.tile([128, N], F32)
        sin_sb = consts.tile([128, N], F32)
        pshuf_sb = consts.tile([128, 128], F32)
        bb_sb = consts.tile([128, C], F32)

        qrot = persist.tile([128, 8, N], BF16)     # Q_rot^T  (d-major, bf16)
        krot = persist.tile([128, 8, N], BF16)
        vext = persist.tile([128, 8, H, 65], BF16)  # V | ones per tok-block
        atok = persist.tile([128, 8, 8, 128], BF16)  # [q, hp, qb, c-in-hp]
        # one whole tile per head-pair: the XBAR transpose requires an
        # offset-0 (whole-tile) output AP
        attn_Ts = [persist.tile([128, 8, 128], BF16, name=f"attn_T{i}")
                   for i in range(8)]
        recip = persist.tile([128, H, 8], F32)       # 1/rowsum per (q, h, qb)
        wp_sb = wpp.tile([128, 8, C], BF16)          # w_proj rows, host-cast

        # ones column of vext (softmax denominator accumulator)
        nc.gpsimd.memset(vext[:, :, :, 64:65], 1.0)

        with tc.tile_pool(name="ep", bufs=4) as ep, \
             tc.tile_pool(name="plg", bufs=2, space="PSUM") as plg, \
             tc.tile_pool(name="pav", bufs=2, space="PSUM") as pav, \
             tc.tile_pool(name="xtp", bufs=1) as xtp, \
             tc.tile_pool(name="wqk", bufs=3) as wqk, \
             tc.tile_pool(name="wvp", bufs=1) as wvp, \
             tc.tile_pool(name="qsb", bufs=2) as qsbp, \
             tc.tile_pool(name="tmp", bufs=2) as tmpp, \
             tc.tile_pool(name="pqk", bufs=2, space="PSUM") as pqk:

            # ---------------- DMA issue (emission order = queue order) ----
            wv_cell = {}

            def fetch_wv(ch):
                # one chunk = 256 V columns = 4 heads
                t = wvp.tile([128, 8, 256], F32, tag="wv", name=f"wv{ch}")
                nc.gpsimd.dma_start(
                    out=f32r(t),
                    in_=f32r(wqkv_h[:, 2048 + ch * 256:2048 + (ch + 1) * 256]
                             .rearrange("(a p) c -> p a c", p=128)))
                wv_cell[ch] = t

            # x^T first in kb halves (the first Q/K matmuls only need kb 0);
            # the V weights follow kb3 so seg0's V work lands mid-segment
            xT = xtp.tile([128, 8, N], F32)

            def fetch_xt(kb):
                nc.gpsimd.dma_start(
                    out=f32r(xT[:, kb, :]),
                    in_=f32r(xt_h[kb * 128:(kb + 1) * 128, :]))

            fetch_xt(0)
            fetch_xt(1)
            fetch_wv(0)
            for kb in range(2, 8):
                fetch_xt(kb)

            wq_ts, wk_ts = {}, {}

            def fetch_wqk(hp):
                for d, store, c0 in ((0, wq_ts, 0), (1, wk_ts, C)):
                    t = wqk.tile([128, 8, 128], F32, tag=f"w{d}",
                                 name=f"w{d}_{hp}")
                    nc.sync.dma_start(
                        out=f32r(t),
                        in_=f32r(wqkv_h[:, c0 + hp * 128:c0 + (hp + 1) * 128]
                                 .rearrange("(a p) c -> p a c", p=128)))
                    store[hp] = t

            fetch_wqk(0)
            nc.sync.dma_start(out=f32r(pshuf_sb), in_=f32r(pshuf_h[:, :]))
            fetch_wqk(1)
            nc.sync.dma_start(out=cos_sb, in_=cos_h[:, :])
            nc.sync.dma_start(out=sin_sb, in_=sin_h[:, :])

            # ---------------- QKV / V emission units ----------------
            def qk_units(hp):
                """Filler units for head-pair hp: per d in (Q, K), per qc
                half: 2 matmul units into a 1-bank PSUM tile, then an
                evac+shuffle+rotate unit."""
                units = []
                for d, wst, dst in ((0, wq_ts, qrot), (1, wk_ts, krot)):
                    cell = {}

                    def mm_u(qc, lo, d=d, wst=wst, cell=cell, hp=hp):
                        if lo == 0:
                            cell[qc] = pqk.tile([128, 512], F32, tag="qk",
                                                name=f"qk{d}_{hp}_{qc}")
                        ps = cell[qc]
                        for kb in range(lo, lo + 2):
                            MM(ps, f32r(wst[hp][:, kb, :]),
                               f32r(xT[:, kb, qc * 512:(qc + 1) * 512]),
                               start=(kb == 0), stop=(kb == 7))

                    def rot_u(qc, d=d, dst=dst, cell=cell, hp=hp):
                        if qc == 0:
                            cell["q"] = qsbp.tile([128, N], F32, tag="q",
                                                  name=f"q{d}_{hp}")
                        q_sb = cell["q"]
                        h0, h1 = qc * 512, (qc + 1) * 512
                        nc.vector.tensor_copy(f32r(q_sb[:, h0:h1]), cell[qc])
                        sh = pqk.tile([128, 512], F32, tag="qk",
                                      name=f"sh{d}_{hp}_{qc}")
                        MM(sh, f32r(pshuf_sb), f32r(q_sb[:, h0:h1]),
                           start=True, stop=True)
                        tm = tmpp.tile([128, 512], F32, tag="t",
                                       name=f"t{d}_{hp}_{qc}")
                        nc.vector.tensor_mul(tm, sh, sin_sb[:, h0:h1])
                        nc.vector.tensor_mul(dst[:, hp, h0:h1],
                                             q_sb[:, h0:h1], cos_sb[:, h0:h1])
                        nc.vector.tensor_add(dst[:, hp, h0:h1],
                                             dst[:, hp, h0:h1], tm)

                    # kb-major across the two qc halves so compute tracks the
                    # x^T chunk arrivals
                    for lo in range(0, 8, 2):
                        for qc in range(2):
                            units.append(lambda qc=qc, lo=lo, f=mm_u: f(qc, lo))
                    units.append(lambda f=rot_u: f(0))
                    units.append(lambda f=rot_u: f(1))
                if hp + 3 < 8:
                    units.append(lambda hp=hp: fetch_wqk(hp + 3))
                return units

            def v_units(ch):
                """V chunk ch (256 cols = heads 4ch..4ch+3), all 8 tok-blocks,
                two [128,1024] lg-pool tiles of 4 tok-blocks each. Each tile's
                whole lifetime (create -> accumulate -> evac) is one atomic
                unit: the lg tag's buffer rotation cannot handle a tile whose
                accumulation interleaves with other tiles of the same tag.
                start=True only for the first region of each PSUM bank."""

                def lump(g, ch=ch):
                    ps = plg.tile([128, N], F32, tag="lg", name=f"v{ch}_{g}")
                    for lo in (0, 2, 4, 6):     # kb-major: tracks x^T arrival
                        for tb4 in range(4):
                            tb = g * 4 + tb4
                            for kb in (lo, lo + 1):
                                MM(ps[:, tb4 * 256:(tb4 + 1) * 256],
                                   f32r(xT[:, kb, tb * 128:(tb + 1) * 128]),
                                   f32r(wv_cell[ch][:, kb, :]),
                                   start=(kb == 0 and tb4 % 2 == 0),
                                   stop=(kb == 7), skip_group_check=True)
                    nc.scalar.copy(
                        vext[:, g * 4:(g + 1) * 4,
                             ch * 4:(ch + 1) * 4, 0:64],
                        ps.rearrange("p (a b c) -> p a b c", a=4, b=4))

                return [lambda g=g, f=lump: f(g) for g in range(2)]

            # ---------------- attention emission ----------------
            def lg_unit(h, kt, estore):
                hp, r0 = h // 2, (h % 2) * 64
                lg_t = plg.tile([128, N], F32, tag="lg", name=f"lg{h}_{kt}")
                for qc in range(2):
                    MM(lg_t[:, qc * 512:(qc + 1) * 512],
                       krot[r0:r0 + 64, hp, kt * 128:(kt + 1) * 128],
                       qrot[r0:r0 + 64, hp, qc * 512:(qc + 1) * 512],
                       start=True, stop=True)
                e_t = ep.tile([128, N], BF16, tag="e", name=f"e{h}_{kt}")
                nc.scalar.activation(e_t, lg_t, EXP, scale=0.125)
                estore[kt] = e_t

            def av_unit(h, kt, estore, av_ts):
                # start=True zeroes the whole PSUM bank: only the first group
                # per bank (qb 0 / 4) may use it; later groups accumulate
                # onto the freshly-zeroed bank.
                e_t = estore[kt]
                for qb in range(8):
                    t = av_ts[qb // 4]
                    j = qb % 4
                    MM(t[:, j * 128:j * 128 + 65],
                       e_t[:, qb * 128:(qb + 1) * 128],
                       vext[:, kt, h, :],
                       start=(kt == 0 and j == 0), stop=(kt == 7),
                       skip_group_check=True)

            def norm_unit(h, av_ts):
                hp, cr0 = h // 2, (h % 2) * 64
                for half in range(2):
                    t = av_ts[half].rearrange("p (a b) -> p a b", a=4)
                    nc.vector.reciprocal(
                        recip[:, h, half * 4:half * 4 + 4], t[:, :, 64])
                for qb in range(8):
                    t = av_ts[qb // 4].rearrange("p (a b) -> p a b", a=4)
                    nc.vector.tensor_scalar(
                        atok[:, hp, qb, cr0:cr0 + 64],
                        t[:, qb % 4, 0:64],
                        recip[:, h, qb:qb + 1], None, MULT)
                if h % 2 == 1:
                    hp = h // 2
                    # tracked guard read: the XBAR transpose's input deps are
                    # not honored; an ordinary DMA on the same SP queue makes
                    # the sequencer wait for the normalize writes first.
                    nc.sync.dma_start(out=guard_h[:, hp * 8:(hp + 1) * 8],
                                      in_=atok[:, hp, 7, 120:128])
                    nc.sync.dma_start(out=attn_Ts[hp],
                                      in_=atok[:, hp, :, :], transpose=True)

            def head_steps(h):
                """Returns (lg_steps, av_steps): 8 of each; av(kt==7) also
                emits the normalize (+ transpose for odd heads)."""
                state = {"e": {}}

                def lg_step(kt, h=h, state=state):
                    lg_unit(h, kt, state["e"])

                def av_step(kt, h=h, state=state):
                    if kt == 0:
                        state["av"] = [
                            pav.tile([128, 512], F32, tag="av",
                                     name=f"av{h}_{i}") for i in range(2)]
                    av_unit(h, kt, state["e"], state["av"])
                    if kt == 7:
                        norm_unit(h, state["av"])

                return ([lambda kt=kt, f=lg_step: f(kt) for kt in range(8)],
                        [lambda kt=kt, f=av_step: f(kt) for kt in range(8)])

            # ---------------- seg0: Q/K(hp0) with V chunk-0 lumps slotted
            # where their weights have landed (x^T kb-major pacing) --------
            lumps = v_units(0)
            qk0 = deque(qk_units(0))
            fetch_wqk(2)
            for i in range(8):
                qk0.popleft()()
            lumps[0]()
            for i in range(8):
                qk0.popleft()()
            lumps[1]()
            while qk0:
                qk0.popleft()()

            # ---------------- segs 1..8: attention + fillers --------------
            lagq = deque()      # av steps lagging 2 behind the lg stream
            fillers = deque()
            fetch_wv(1)
            for seg in range(1, 9):
                if seg < 8:
                    fillers.extend(qk_units(seg))
                if seg in (1, 2, 4):
                    # V chunks 1-3 (heads 4-15); chunk c needed first by
                    # attention head 4c (seg 2c+1)
                    fillers.extend(v_units(seg if seg < 4 else 3))
                    if seg < 4:
                        fetch_wv(seg + 1)
                if seg == 6:
                    # prefetch proj weights + bias on the now-quiet SP queue
                    for cb in range(8):
                        nc.sync.dma_start(
                            out=wp_sb[:, cb, :],
                            in_=wpb_h[cb * 128:(cb + 1) * 128, :])
                    nc.sync.dma_start(out=bb_sb, in_=bb_h[:, :])
                steps_left = 16
                for h in (2 * (seg - 1), 2 * (seg - 1) + 1):
                    lg_steps, av_steps = head_steps(h)
                    for kt in range(8):
                        lg_steps[kt]()
                        lagq.append(av_steps[kt])
                        if len(lagq) > 2:
                            lagq.popleft()()
                        # pace fillers evenly across the segment
                        budget = 2 if len(fillers) > steps_left else 1
                        for _ in range(budget):
                            if fillers:
                                fillers.popleft()()
                        steps_left -= 1
            while lagq:
                lagq.popleft()()

        # ---------------- proj + bias + output ----------------
        with tc.tile_pool(name="ysb", bufs=2) as ysbp, \
             tc.tile_pool(name="psy", bufs=2, space="PSUM") as psy:
            for tb in range(8):
                y_ps = psy.tile([128, C], F32, tag="y", name=f"y{tb}")
                for cb in range(8):
                    for oc in range(2):
                        MM(y_ps[:, oc * 512:(oc + 1) * 512],
                           attn_Ts[cb][:, tb, :],
                           wp_sb[:, cb, oc * 512:(oc + 1) * 512],
                           start=(cb == 0), stop=(cb == 7))
                y_sb = ysbp.tile([128, C], F32, tag="ysb", name=f"ysb{tb}")
                nc.vector.tensor_add(y_sb, y_ps, bb_sb)
                nc.sync.dma_start(out=out_h[tb * 128:(tb + 1) * 128, :],
                                  in_=y_sb)

    nc.finalize()
    return nc


_PROGRAM = None


def kernel(x, w_qkv, w_proj, b_proj):
    global _PROGRAM
    if _PROGRAM is None:
        _PROGRAM = _build_program()
    nc = _PROGRAM

    import ml_dtypes

    cos_d, sin_d, pshuf = _host_tables()
    shared = {
        "w_qkv": np.ascontiguousarray(w_qkv, np.float32),
        "w_projb": np.ascontiguousarray(
            np.asarray(w_proj, np.float32).astype(ml_dtypes.bfloat16)
        ).view(np.uint16),
        "b_bcast": np.ascontiguousarray(
            np.broadcast_to(np.asarray(b_proj, np.float32).reshape(1, C),
                            (128, C))),
        "cos_d": cos_d,
        "sin_d": sin_d,
        "pshuf": pshuf,
    }
    in_maps = [
        {"xt": np.ascontiguousarray(np.asarray(x[b], np.float32).T), **shared}
        for b in range(NCORES)
    ]
    res = run_bass_kernel_spmd(nc, in_maps, core_ids=list(range(NCORES)))
    return np.stack([res.results[b]["out"] for b in range(NCORES)], axis=0)


if __name__ == "__main__":
    xs = np.random.randn(B, N, C).astype(np.float32)
    wq = (np.random.randn(C, 3 * C) / np.sqrt(C)).astype(np.float32)
    wp = (np.random.randn(C, C) / np.sqrt(C)).astype(np.float32)
    bp = (np.random.randn(C) * 0.01).astype(np.float32)
    out = kernel(x=xs, w_qkv=wq, w_proj=wp, b_proj=bp)
    print(out.shape, out.dtype)
